# revision 17
# baseline (speedup 1.0000x reference)
"""Self-contained Trainium2 Bass kernel for nn_Model_16801912062040 (dense_cnn).

Sharding: batch-parallel, 2 samples per core across 8 cores, ZERO collectives.
The dynamic conv collapses algebraically: y[n,(m,o)] = alpha[m,o]*(x2[n] (x) Wi[o])
and the conv_transpose contraction over (m,o) reduces to
    z[n,i] = sum_o s_o * (Y0[n,o] (x)_full flip(Wi[o,i])),  s_o = sum_m alpha[m,o]^2
so only the per-channel scalar s (128 floats per layer) couples samples.

Every core computes s for BOTH layers locally from the full x: all pooled
statistics are linear in x up to the alpha nonlinearity, and the raster reshape
(B,L,D)->(B*D,1,L) has an exact shift structure in nt (1536 flat elements = 12
l-steps), so pp0 and the Y0 row-sums (ry0) for all 16 samples are stride-12
convs of x with host-composed kernels (G0: 14 taps; G2: 38 taps folding Wi and
the windowed-column-sum edge corrections); row-sums of z then come from 3
matmuls against row-flipped row-summed Wi (wrf), giving pp1 and s1. Each core
is fully independent: no barriers, no skew amplification.

Perf notes:
- fp32 matmuls run ~4x slower per column than bf16 -> every matmul input is
  bf16 (PSUM accumulation stays f32).
- Each dma_start costs ~700ns serialized on its engine's sequencer, so params
  are packed into 6 loads spread across different engine queues.
- The stats pipeline is interleaved between conv blocks so its cross-engine
  latency hides behind matmul streams instead of stalling the PE queue.
- The psum->x2 gather is 6 strided AP copies instead of 24 per sample.
"""
import math
import numpy as np

import concourse.bass as bass
import concourse.tile as tile
from concourse import bacc, mybir
from concourse.bass_utils import run_bass_kernel_spmd

N_CORES = 8
BPC = 2            # samples per core
B, L, CIN, D = 16, 192, 21, 128
P, S, NT, PRED, COUT = 24, 12, 16, 96, 21
LAYERS = 2
BN_EPS = 1e-5
F32 = mybir.dt.float32
BF16 = mybir.dt.bfloat16

# pack -> (partitions, dtype, [(piece, piece_partitions, cols), ...])
PACKS = {
    "pkm": (128, "bf16", [("posT", 128, 192), ("posR", 128, 192),
                          ("w2pa", 128, 384), ("w2pb", 65, 384)]),
    "pkw": (128, "bf16", [("wiT", 128, 2304), ("w2T", 128, 2304)]),
    "pkt": (21, "bf16", [("tokA", 21, 384), ("tokR", 21, 384)]),
    "pkc": (128, "bf16", [("bns", 128, 2), ("bnb", 128, 2),
                          ("cpp0r", 128, 256), ("c2r", 128, 224),
                          ("pcvT", 128, 768), ("awT", 128, 256),
                          ("wrf", 128, 384), ("fc1e", 128, 288),
                          ("fc2T", 128, 21), ("g0T", 98, 384), ("xg", 98, 768),
                          ("g2T", 114, 896), ("xg2", 114, 1568),
                          ("fc1b", 1, 96), ("fc2b", 1, 21)]),
}

_CACHE = {}
LAST_RESULT = None


def _pos_embed():
    pos = np.arange(L, dtype=np.float32)[:, None]
    div = np.exp(np.arange(0, D, 2, dtype=np.float32) * (-math.log(10000.0) / D))
    pe = np.zeros((L, D), np.float32)
    pe[:, 0::2] = np.sin(pos * div)
    pe[:, 1::2] = np.cos(pos * div)
    return pe


def _slice_map(p, patch_w2, patch_b):
    """Phi/weight/bias map for xe4[:, nt, p, d_t] in E-flat space (nt-shift-free)."""
    d_t = np.arange(D)
    g = p // 3
    inner = (p % 3) * 128 + d_t
    nts = inner // 24
    ps = inner % 24
    k = np.arange(P)
    j = np.minimum(12 * nts[:, None] + k[None, :], 191)   # edge-pad fold
    Phi = 192 * g + j
    return Phi, patch_w2[ps], patch_b[ps]


def _density(ps_list, patch_w2, patch_b):
    d_t = np.arange(D)
    dens = np.zeros((D, 12, 128), np.float32)
    bias = np.zeros(D, np.float32)
    for p in ps_list:
        Phi, w, b = _slice_map(p, patch_w2, patch_b)
        for k in range(P):
            np.add.at(dens, (d_t, Phi[:, k] // 128, Phi[:, k] % 128), w[:, k])
        bias += b
    return dens, bias


def _compose_G(dens, bias, token_w, pe_t):
    G = np.zeros((D, CIN, 14), np.float32)
    for k2 in range(3):
        G[:, :, k2:k2 + 12] += np.einsum("dlm,mc->dcl", dens, token_w[:, :, k2])
    C = np.einsum("dlm,tlm->dt", dens, pe_t) + bias[:, None]
    return G, C


def _prep_consts(token_w, patch_w, patch_b, Wi, pconv_w, pconv_b, bn_g, bn_b,
                 aconv_w, fc1_w, fc1_b, fc2_w, fc2_b):
    c = {}
    c["tokA"] = token_w.transpose(1, 2, 0).reshape(CIN, -1)
    c["tokR"] = np.roll(token_w, -64, 0).transpose(1, 2, 0).reshape(CIN, -1)
    pe = _pos_embed()
    c["posT"] = pe.T
    c["posR"] = np.ascontiguousarray(np.roll(pe.T, -64, 0))
    W2p = np.zeros((205, 384), np.float32)
    for nt in range(NT):
        for p_ in range(P):
            j = nt * 24 + p_
            for k in range(P):
                m = 12 * nt + k
                W2p[min(m, 191), j] += patch_w[p_, 0, k]   # replicate-pad fold
            W2p[204, j] = patch_b[p_]
    c["w2pa"] = W2p[0:128]
    c["w2pb"] = np.concatenate([W2p[128:192], W2p[204:205]], 0)
    A = Wi.transpose(0, 3, 4, 2, 1).reshape(LAYERS, 9, D, D)                  # [l,tap,i,o]
    c["wiT"] = A.transpose(2, 0, 1, 3).reshape(D, LAYERS * 9 * D)
    Wf = Wi[:, :, :, ::-1, ::-1]
    Bt = Wf.transpose(0, 3, 4, 1, 2).reshape(LAYERS, 9, D, D)                 # [l,tap,o,i]
    c["w2T"] = Bt.transpose(2, 0, 1, 3).reshape(D, LAYERS * 9 * D)
    Ct = pconv_w.transpose(0, 3, 2, 1) / 24.0
    c["pcvT"] = Ct.transpose(2, 0, 1, 3).reshape(D, LAYERS * 3 * D)
    c["awT"] = (aconv_w.transpose(2, 0, 1) / 16.0).reshape(D, LAYERS * D)
    c["bns"] = (bn_g / np.sqrt(1.0 + BN_EPS)).T
    c["fc1e"] = fc1_w.T.reshape(D, 3 * PRED)
    c["fc1b"] = fc1_b[None, :]
    c["fc2T"] = fc2_w.T
    c["fc2b"] = fc2_b[None, :]
    c["bnb"] = (pconv_b * (bn_g / np.sqrt(1.0 + BN_EPS)) + bn_b).T

    # ---- composed all-sample stats kernels (no collectives) ----
    patch_w2 = patch_w[:, 0, :]
    pe_t = pe[(12 * np.arange(NT)[:, None] + np.arange(12)[None, :]).reshape(-1)].reshape(NT, 12, D)
    dens0, bias0 = _density(range(P), patch_w2, patch_b)
    G0, C0 = _compose_G(dens0, bias0, token_w, pe_t)
    dxe = {p: _density([p], patch_w2, patch_b) for p in (0, 2, 21, 23)}
    cxd = [
        (dens0 - dxe[21][0] - dxe[23][0], bias0 - dxe[21][1] - dxe[23][1]),
        (dens0 - dxe[0][0] - dxe[23][0], bias0 - dxe[0][1] - dxe[23][1]),
        (dens0 - dxe[0][0] - dxe[2][0], bias0 - dxe[0][1] - dxe[2][1]),
    ]
    G2 = np.zeros((D, CIN, 38), np.float32)
    C2 = np.zeros((D, 14), np.float32)
    for dq in range(3):
        Gcx, Ccx = _compose_G(cxd[dq][0], cxd[dq][1], token_w, pe_t)
        for dp in range(3):
            Wt = Wi[0][:, :, dp, dq]
            G2[:, :, 12 * dp:12 * dp + 14] += np.einsum("oi,icj->ocj", Wt, Gcx)
            C2 += np.einsum("oi,it->ot", Wt, Ccx[:, dp:dp + 14])
    G0T = G0.reshape(D, CIN * 14).T
    c["g0T"] = np.stack([G0T[98 * i:98 * (i + 1)] for i in range(3)], 1).reshape(98, 3 * 128)
    G2T = G2.reshape(D, CIN * 38).T
    c["g2T"] = np.stack([G2T[114 * i:114 * (i + 1)] for i in range(7)], 1).reshape(114, 7 * 128)
    c["cpp0r"] = np.broadcast_to(C0[:, None, :], (D, B, NT)).reshape(D, -1)
    c["c2r"] = np.broadcast_to(C2[:, None, :], (D, B, 14)).reshape(D, -1)
    wrfn = Wi[0][:, :, ::-1, :].sum(-1)                                       # [o, i, dp]
    c["wrf"] = wrfn.transpose(0, 2, 1).reshape(D, 3 * D)
    return c


def _build():
    nc = bacc.Bacc("TRN2", target_bir_lowering=False, debug=False, num_devices=N_CORES)

    xtp = nc.declare_dram_parameter("xtp", [CIN, BPC * 194], BF16, isOutput=False)
    pk_params = {}
    for pname, (parts, dt, pieces) in PACKS.items():
        total = sum(w for _, _, w in pieces)
        pk_params[pname] = nc.declare_dram_parameter(
            pname, [parts, total], F32 if dt == "f32" else BF16, isOutput=False)
    out = nc.declare_dram_parameter("out", [BPC, PRED, COUT], F32, isOutput=True)

    RELU = mybir.ActivationFunctionType.Relu
    SQUARE = mybir.ActivationFunctionType.Square
    ADD = mybir.AluOpType.add
    AX = mybir.AxisListType.X

    with tile.TileContext(nc) as tc:
        with tc.tile_pool(name="w", bufs=1) as wp, \
             tc.tile_pool(name="act", bufs=2) as ap, \
             tc.tile_pool(name="x2p", bufs=6) as xp, \
             tc.tile_pool(name="ps", bufs=1, space="PSUM") as pp:

            # --- packed param loads, triggers spread across engine queues ---
            pieces = {}
            dma_eng = {"pkt": nc.sync, "pkm": nc.scalar}
            xt_sb = wp.tile([CIN, BPC * 194], BF16, tag="xt")
            nc.sync.dma_start(out=xt_sb[:], in_=xtp[:, :])
            for pname in ("pkt", "pkm", "pkc", "pkw"):
                # pkc/pkw triggers are issued after the embedding block
                parts, dt, plist = PACKS[pname]
                total = sum(w for _, _, w in plist)
                t = wp.tile([parts, total], F32 if dt == "f32" else BF16, tag=pname)
                if pname not in ("pkw", "pkc"):
                    dma_eng[pname].dma_start(out=t[:], in_=pk_params[pname][:, :])
                o = 0
                for nm, pparts, w_ in plist:
                    pieces[nm] = (t, pparts, o)
                    o += w_
            pkw_tile = pieces["wiT"][0]
            pkc_tile = pieces["g0T"][0]
            delayed = False

            def pv(nm, a, b_):
                t, pparts, o = pieces[nm]
                return t[0:pparts, o + a:o + b_]

            bnsb = wp.tile([D, 4], F32, tag="bnsb")
            ones_sb = wp.tile([1, D], BF16, tag="ones")
            nc.vector.memset(ones_sb[:], 1.0)
            ryspad = wp.tile([D, B, 18], BF16, tag="ryspad")
            nc.vector.memset(ryspad[:], 0.0)
            y0pads = {}
            for l in range(LAYERS):
                for n in range(BPC):
                    y0p = ap.tile([D, 18, 26], BF16, tag="y0p", bufs=4)
                    nc.vector.memset(y0p[:], 0.0)   # borders only matter
                    y0pads[(l, n)] = y0p

            # ---------------- embedding (own 2 samples) ----------------
            x2 = []
            for n in range(BPC):
                embs = []
                for tok, pos in (("tokA", "posT"), ("tokR", "posR")):
                    e_ps = pp.tile([D, L], F32, tag="ps", bufs=6)
                    for k in range(3):
                        nc.tensor.matmul(e_ps[:], lhsT=pv(tok, k * D, (k + 1) * D),
                                         rhs=xt_sb[:, 194 * n + k:194 * n + k + L],
                                         start=(k == 0), stop=(k == 2))
                    e_sb = ap.tile([D, L], F32, tag="emb_sb", bufs=2)
                    nc.vector.tensor_add(out=e_sb[:], in0=e_ps[:], in1=pv(pos, 0, L))
                    embs.append(e_sb)
                eT = embs[0][:].rearrange("p (s c) -> p s c", c=3)
                eR = embs[1][:].rearrange("p (s c) -> p s c", c=3)
                xeA = ap.tile([D, 64, 2], BF16, tag="xeA")
                xeB = ap.tile([65, 64, 2], BF16, tag="xeB")
                nc.vector.tensor_copy(out=xeA[:, :, 0], in_=eT[:, :, 0])
                nc.vector.tensor_copy(out=xeA[0:64, :, 1], in_=eR[0:64, :, 1])
                nc.vector.tensor_copy(out=xeA[64:128, :, 1], in_=eR[64:128, :, 2])
                nc.scalar.copy(out=xeB[0:64, :, 0], in_=eT[0:64, :, 1])
                nc.scalar.copy(out=xeB[0:64, :, 1], in_=eR[0:64, :, 2])
                nc.vector.memset(xeB[64:65, :, :], 1.0)
                if not delayed:
                    delayed = True
                    # WAW corner-writes hold the big packs off the DMA rings
                    # until the urgent embedding loads have drained
                    nc.scalar.copy(out=pkc_tile[0:1, 0:1], in_=xeA[0:1, 0:1, 0])
                    nc.scalar.copy(out=pkw_tile[0:1, 0:1], in_=xeA[0:1, 0:1, 0])
                    nc.scalar.dma_start(out=pkc_tile[:], in_=pk_params["pkc"][:, :])
                    nc.scalar.dma_start(out=pkw_tile[:], in_=pk_params["pkw"][:, :])
                pcs = []
                for e in range(3):
                    pc_ps = pp.tile([D, NT, 8], F32, tag="ps", bufs=6)
                    nc.tensor.matmul(pc_ps[:], lhsT=pv("w2pa", 128 * e, 128 * (e + 1)),
                                     rhs=xeA[:], start=True, stop=False)
                    nc.tensor.matmul(pc_ps[:], lhsT=pv("w2pb", 128 * e, 128 * (e + 1)),
                                     rhs=xeB[:], start=False, stop=True)
                    pcs.append(pc_ps)
                # psum -> x2 permutation as 6 strided AP copies:
                # x2 w = 3k+r <-> pcs[e] q = 2k2+r2 with p(w) = 2*(w%12) + w//12
                x2n = xp.tile([D, NT, 24], F32, tag="x2")
                x2v = x2n[:].rearrange("p t (k r) -> p t k r", r=3)
                plan = [(0, 0, 0, 0), (0, 1, 1, 1),
                        (1, 2, 0, 1), (1, 0, 1, 0),
                        (2, 1, 0, 0), (2, 2, 1, 1)]
                for i, (e, r, half, r2) in enumerate(plan):
                    pcv = pcs[e][:].rearrange("p t (k q) -> p t k q", q=2)
                    eng = nc.vector.tensor_copy if i % 2 == 0 else (
                        lambda out, in_: nc.scalar.copy(out=out, in_=in_))
                    eng(out=x2v[:, :, 4 * half:4 * (half + 1), r],
                        in_=pcv[:, :, :, r2])
                x2.append(x2n)

            nc.vector.tensor_copy(out=bnsb[:], in_=pv("bns", 0, 4))

            # ------- stats pipeline pieces (interleaved with conv blocks) -------
            def ppc_pool(l, ppsb):
                ppc_ps = pp.tile([D, B, NT], F32, tag="st", bufs=2)
                for k in range(3):
                    o = (l * 3 + k) * D
                    if k == 0:
                        nc.tensor.matmul(ppc_ps[:, :, 1:NT], lhsT=pv("pcvT", o, o + D),
                                         rhs=ppsb[:, :, 0:NT - 1], start=True, stop=False)
                    elif k == 1:
                        nc.tensor.matmul(ppc_ps[:], lhsT=pv("pcvT", o, o + D),
                                         rhs=ppsb[:], start=False, stop=False)
                    else:
                        nc.tensor.matmul(ppc_ps[:, :, 0:NT - 1], lhsT=pv("pcvT", o, o + D),
                                         rhs=ppsb[:, :, 1:NT], start=False, stop=True)
                ppc_sb = ap.tile([D, B, NT], F32, tag="ppc_sb", bufs=2)
                nc.scalar.activation(out=ppc_sb[:], in_=ppc_ps[:], func=RELU,
                                     bias=bnsb[:, 2 + l:3 + l], scale=bnsb[:, l:l + 1])
                pooled = ap.tile([D, B], BF16, tag="pooled", bufs=2)
                with nc.allow_low_precision(reason="16-term pooled mean, tol 2e-2"):
                    nc.vector.tensor_reduce(out=pooled[:], in_=ppc_sb[:], axis=AX, op=ADD)
                return pooled

            def alpha_sq(l, pooled):
                al_ps = pp.tile([D, B], F32, tag="st", bufs=2)
                nc.tensor.matmul(al_ps[:], lhsT=pv("awT", l * D, (l + 1) * D),
                                 rhs=pooled[:], start=True, stop=True)
                asq = ap.tile([D, B], F32, tag="asq", bufs=2)
                s_sb = ap.tile([D, 1], F32, tag="s", bufs=2)
                nc.scalar.activation(out=asq[:], in_=al_ps[:], func=SQUARE,
                                     bias=1.0, scale=1.0, accum_out=s_sb[:])
                return s_sb

            def conv9(out_ps, wname, base, rhs_tile, windows):
                for dp in range(3):
                    for dq in range(3):
                        tap = 3 * dp + dq
                        o = base + tap * D
                        nc.tensor.matmul(out_ps[:], lhsT=pv(wname, o, o + D),
                                         rhs=rhs_tile[:, dp:dp + windows[0],
                                                      dq:dq + windows[1]],
                                         start=(tap == 0), stop=(tap == 8))

            # stats phase 1: pp0, ry0, ppc0 -> pooled0
            pp0_ps = pp.tile([D, B, NT], F32, tag="st", bufs=2)
            for cc in range(3):
                nc.tensor.matmul(pp0_ps[:], lhsT=pv("g0T", 128 * cc, 128 * (cc + 1)),
                                 rhs=pv("xg", 256 * cc, 256 * (cc + 1)).rearrange(
                                     "p (b t) -> p b t", t=NT),
                                 start=(cc == 0), stop=(cc == 2))
            pp0_sb = wp.tile([D, B, NT], BF16, tag="pp0")
            nc.vector.tensor_add(out=pp0_sb[:], in0=pp0_ps[:],
                                 in1=pv("cpp0r", 0, 256).rearrange("p (b t) -> p b t", t=NT))
            ry0_ps = pp.tile([D, B, 14], F32, tag="st", bufs=2)
            for cc in range(7):
                nc.tensor.matmul(ry0_ps[:], lhsT=pv("g2T", 128 * cc, 128 * (cc + 1)),
                                 rhs=pv("xg2", 224 * cc, 224 * (cc + 1)).rearrange(
                                     "p (b r) -> p b r", r=14),
                                 start=(cc == 0), stop=(cc == 6))
            ry0_sb = ap.tile([D, B, 14], F32, tag="ry0")
            nc.vector.tensor_add(out=ry0_sb[:], in0=ry0_ps[:],
                                 in1=pv("c2r", 0, 224).rearrange("p (b r) -> p b r", r=14))
            pooled0 = ppc_pool(0, pp0_sb)

            # conv l=0 part A: casts + Y0
            x2b = []
            for n in range(BPC):
                x2bn = ap.tile([D, NT, 24], BF16, tag="x2b", bufs=2)
                (nc.vector.tensor_copy if n == 0 else
                 (lambda out, in_: nc.scalar.copy(out=out, in_=in_)))(
                    out=x2bn[:], in_=x2[n][:])
                x2b.append(x2bn)
            y0ps = []
            for n in range(BPC):
                y0_ps = pp.tile([D, 14, 22], F32, tag="ps", bufs=6)
                conv9(y0_ps, "wiT", 0, x2b[n], (14, 22))
                y0ps.append(y0_ps)

            # stats phase 2: s0, rsz -> pp1 -> pooled1
            s0_sb = alpha_sq(0, pooled0)
            nc.vector.tensor_scalar_mul(out=ryspad[:, :, 2:16], in0=ry0_sb[:],
                                        scalar1=s0_sb[:])
            rsz_ps = pp.tile([D, B, NT], F32, tag="st", bufs=2)
            for dp in range(3):
                nc.tensor.matmul(rsz_ps[:], lhsT=pv("wrf", dp * D, (dp + 1) * D),
                                 rhs=ryspad[:, :, dp:dp + 16],
                                 start=(dp == 0), stop=(dp == 2))
            pp1_sb = wp.tile([D, B, NT], BF16, tag="pp1")
            nc.vector.tensor_add(out=pp1_sb[:], in0=rsz_ps[:], in1=pp0_sb[:])
            pooled1 = ppc_pool(1, pp1_sb)

            # conv l=0 part B: scale + z + residual
            for n in range(BPC):
                y0p = y0pads[(0, n)]
                nc.vector.tensor_scalar_mul(out=y0p[:, 2:16, 2:24], in0=y0ps[n][:],
                                            scalar1=s0_sb[:])
                z_ps = pp.tile([D, NT, 24], F32, tag="ps", bufs=6)
                conv9(z_ps, "w2T", 0, y0p, (16, 24))
                x2n = xp.tile([D, NT, 24], F32, tag="x2")
                nc.vector.tensor_add(out=x2n[:], in0=z_ps[:], in1=x2[n][:])
                x2[n] = x2n

            s1_sb = alpha_sq(1, pooled1)

            # conv l=1
            x2b = []
            for n in range(BPC):
                x2bn = ap.tile([D, NT, 24], BF16, tag="x2b", bufs=2)
                (nc.vector.tensor_copy if n == 0 else
                 (lambda out, in_: nc.scalar.copy(out=out, in_=in_)))(
                    out=x2bn[:], in_=x2[n][:])
                x2b.append(x2bn)
            y0ps = []
            for n in range(BPC):
                y0_ps = pp.tile([D, 14, 22], F32, tag="ps", bufs=6)
                conv9(y0_ps, "wiT", 9 * D, x2b[n], (14, 22))
                y0ps.append(y0_ps)
            x2fin = []
            for n in range(BPC):
                y0p = y0pads[(1, n)]
                nc.vector.tensor_scalar_mul(out=y0p[:, 2:16, 2:24], in0=y0ps[n][:],
                                            scalar1=s1_sb[:])
                z_ps = pp.tile([D, NT, 24], F32, tag="ps", bufs=6)
                conv9(z_ps, "w2T", 9 * D, y0p, (16, 24))
                x2h = ap.tile([D, NT, 24], BF16, tag="x2h", bufs=2)
                with nc.allow_low_precision(reason="head input, tol 2e-2"):
                    nc.vector.tensor_add(out=x2h[:], in0=z_ps[:], in1=x2[n][:])
                x2fin.append(x2h)

            # ---------------- heads ----------------
            o_all = ap.tile([PRED, BPC, COUT], F32, tag="oall")
            for n in range(BPC):
                x2f = x2fin[n][:].rearrange("p a b -> p (a b)")
                y1_ps = pp.tile([D, PRED], F32, tag="ps", bufs=6)
                for e in range(3):
                    nc.tensor.matmul(y1_ps[:], lhsT=x2f[:, 128 * e:128 * (e + 1)],
                                     rhs=pv("fc1e", PRED * e, PRED * (e + 1)),
                                     start=(e == 0), stop=False)
                nc.tensor.matmul(y1_ps[:], lhsT=ones_sb[:], rhs=pv("fc1b", 0, PRED),
                                 start=False, stop=True)
                y1_sb = ap.tile([D, PRED], BF16, tag="y1sb")
                nc.scalar.copy(out=y1_sb[:], in_=y1_ps[:])
                o_ps = pp.tile([PRED, COUT], F32, tag="ps", bufs=6)
                nc.tensor.matmul(o_ps[:], lhsT=y1_sb[:], rhs=pv("fc2T", 0, COUT),
                                 start=True, stop=False)
                nc.tensor.matmul(o_ps[:], lhsT=ones_sb[:, 0:PRED], rhs=pv("fc2b", 0, COUT),
                                 start=False, stop=True)
                (nc.vector.tensor_copy if n == 0 else
                 (lambda out, in_: nc.scalar.copy(out=out, in_=in_)))(
                    out=o_all[:, n, :], in_=o_ps[:])
            nc.sync.dma_start(out=out[:, :, :].rearrange("n p c -> p n c"),
                              in_=o_all[:])

    nc.finalize()
    return nc


def kernel(**inputs):
    global LAST_RESULT
    import ml_dtypes
    inputs = {k: np.ascontiguousarray(np.asarray(v, np.float32)) for k, v in inputs.items()}
    if "nc" not in _CACHE:
        _CACHE["nc"] = _build()
    nc = _CACHE["nc"]
    c = _prep_consts(
        inputs["token_w"], inputs["patch_w"], inputs["patch_b"], inputs["Wi"],
        inputs["pconv_w"], inputs["pconv_b"], inputs["bn_g"], inputs["bn_b"],
        inputs["aconv_w"], inputs["fc1_w"], inputs["fc1_b"], inputs["fc2_w"],
        inputs["fc2_b"])
    xtp_full = np.pad(inputs["x"].transpose(0, 2, 1), ((0, 0), (0, 0), (1, 1)),
                      mode="wrap").astype(np.float32)
    # im2col gathers of x for the stats path (identical on all cores)
    xG = np.empty((CIN, 14, B, NT), np.float32)
    for jp in range(14):
        xG[:, jp] = xtp_full[:, :, jp::12][:, :, :NT].transpose(1, 0, 2)
    xG = xG.reshape(CIN * 14, B * NT)
    c["xg"] = np.stack([xG[98 * i:98 * (i + 1)] for i in range(3)], 1).reshape(98, 3 * 256)
    xG2 = np.empty((CIN, 38, B, 14), np.float32)
    for jp in range(38):
        xG2[:, jp] = xtp_full[:, :, jp::12][:, :, :14].transpose(1, 0, 2)
    xG2 = xG2.reshape(CIN * 38, B * 14)
    c["xg2"] = np.stack([xG2[114 * i:114 * (i + 1)] for i in range(7)], 1).reshape(114, 7 * 224)

    base = {}
    for pname, (parts, dt, plist) in PACKS.items():
        cols = []
        for nm, pparts, w_ in plist:
            a = np.zeros((parts, w_), np.float32)
            a[:pparts] = np.asarray(c[nm], np.float32).reshape(pparts, w_)
            cols.append(a)
        arr = np.concatenate(cols, axis=1)
        base[pname] = np.ascontiguousarray(
            arr.astype(ml_dtypes.bfloat16 if dt == "bf16" else np.float32))
    in_maps = []
    for core in range(N_CORES):
        m = dict(base)
        xt = np.concatenate([xtp_full[BPC * core + n] for n in range(BPC)], axis=1)
        m["xtp"] = np.ascontiguousarray(xt.astype(ml_dtypes.bfloat16))
        in_maps.append(m)
    import os
    res = run_bass_kernel_spmd(nc, in_maps, core_ids=list(range(N_CORES)),
                               trace=bool(os.environ.get("BASS_TRACE")))
    LAST_RESULT = res
    return np.concatenate([res.results[cid]["out"] for cid in range(N_CORES)], axis=0)


# revision 19
# speedup vs baseline: 1.0512x; 1.0512x over previous
"""Self-contained Trainium2 Bass kernel for nn_Model_16801912062040 (dense_cnn).

Sharding: batch-parallel, 2 samples per core across 8 cores, ZERO collectives.
The dynamic conv collapses algebraically: y[n,(m,o)] = alpha[m,o]*(x2[n] (x) Wi[o])
and the conv_transpose contraction over (m,o) reduces to
    z[n,i] = sum_o s_o * (Y0[n,o] (x)_full flip(Wi[o,i])),  s_o = sum_m alpha[m,o]^2
so only the per-channel scalar s (128 floats per layer) couples samples.

Every core computes s for BOTH layers locally from the full x: all pooled
statistics are linear in x up to the alpha nonlinearity, and the raster reshape
(B,L,D)->(B*D,1,L) has an exact shift structure in nt (1536 flat elements = 12
l-steps), so pp0 and the Y0 row-sums (ry0) for all 16 samples are stride-12
convs of x with host-composed kernels (G0: 14 taps; G2: 38 taps folding Wi and
the windowed-column-sum edge corrections); row-sums of z then come from 3
matmuls against row-flipped row-summed Wi (wrf), giving pp1 and s1. Each core
is fully independent: no barriers, no skew amplification.

Perf notes:
- fp32 matmuls run ~4x slower per column than bf16 -> every matmul input is
  bf16 (PSUM accumulation stays f32).
- Each dma_start costs ~700ns serialized on its engine's sequencer, so params
  are packed into 6 loads spread across different engine queues.
- The stats pipeline is interleaved between conv blocks so its cross-engine
  latency hides behind matmul streams instead of stalling the PE queue.
- The psum->x2 gather is 6 strided AP copies instead of 24 per sample.
"""
import math
import numpy as np

import concourse.bass as bass
import concourse.tile as tile
from concourse import bacc, mybir
from concourse.bass_utils import run_bass_kernel_spmd

N_CORES = 8
BPC = 2            # samples per core
B, L, CIN, D = 16, 192, 21, 128
P, S, NT, PRED, COUT = 24, 12, 16, 96, 21
LAYERS = 2
BN_EPS = 1e-5
F32 = mybir.dt.float32
BF16 = mybir.dt.bfloat16

# pack -> (partitions, dtype, [(piece, piece_partitions, cols), ...])
PACKS = {
    "pkm": (128, "bf16", [("posx2", 128, 384),
                          ("w2pa", 128, 384), ("w2pb", 65, 384)]),
    "pkw": (128, "bf16", [("wiT", 128, 2304), ("w2T", 128, 2304)]),
    "pkt": (21, "bf16", [("tokA", 21, 384), ("tokR", 21, 384)]),
    "pkc": (128, "bf16", [("bns", 128, 2), ("bnb", 128, 2),
                          ("cpp0r", 128, 256), ("c2r", 128, 224),
                          ("pcvT", 128, 768), ("awT", 128, 256),
                          ("wrf", 128, 384), ("fc1e", 128, 288),
                          ("fc2T", 128, 21), ("g0T", 98, 384), ("xg", 98, 768),
                          ("g2T", 114, 896), ("xg2", 114, 1568),
                          ("fc1b", 1, 96), ("fc2b", 1, 21)]),
}

_CACHE = {}
LAST_RESULT = None


def _pos_embed():
    pos = np.arange(L, dtype=np.float32)[:, None]
    div = np.exp(np.arange(0, D, 2, dtype=np.float32) * (-math.log(10000.0) / D))
    pe = np.zeros((L, D), np.float32)
    pe[:, 0::2] = np.sin(pos * div)
    pe[:, 1::2] = np.cos(pos * div)
    return pe


def _slice_map(p, patch_w2, patch_b):
    """Phi/weight/bias map for xe4[:, nt, p, d_t] in E-flat space (nt-shift-free)."""
    d_t = np.arange(D)
    g = p // 3
    inner = (p % 3) * 128 + d_t
    nts = inner // 24
    ps = inner % 24
    k = np.arange(P)
    j = np.minimum(12 * nts[:, None] + k[None, :], 191)   # edge-pad fold
    Phi = 192 * g + j
    return Phi, patch_w2[ps], patch_b[ps]


def _density(ps_list, patch_w2, patch_b):
    d_t = np.arange(D)
    dens = np.zeros((D, 12, 128), np.float32)
    bias = np.zeros(D, np.float32)
    for p in ps_list:
        Phi, w, b = _slice_map(p, patch_w2, patch_b)
        for k in range(P):
            np.add.at(dens, (d_t, Phi[:, k] // 128, Phi[:, k] % 128), w[:, k])
        bias += b
    return dens, bias


def _compose_G(dens, bias, token_w, pe_t):
    G = np.zeros((D, CIN, 14), np.float32)
    for k2 in range(3):
        G[:, :, k2:k2 + 12] += np.einsum("dlm,mc->dcl", dens, token_w[:, :, k2])
    C = np.einsum("dlm,tlm->dt", dens, pe_t) + bias[:, None]
    return G, C


def _prep_consts(token_w, patch_w, patch_b, Wi, pconv_w, pconv_b, bn_g, bn_b,
                 aconv_w, fc1_w, fc1_b, fc2_w, fc2_b):
    c = {}
    c["tokA"] = token_w.transpose(1, 2, 0).reshape(CIN, -1)
    c["tokR"] = np.roll(token_w, -64, 0).transpose(1, 2, 0).reshape(CIN, -1)
    pe = _pos_embed()
    pos_flat = pe.reshape(L * D)
    xe4_pos = np.zeros((NT, P, D), np.float32)
    for p_ in range(P):
        Phi, w, _ = _slice_map(p_, patch_w[:, 0, :], patch_b)
        for nt in range(NT):
            xe4_pos[nt, p_] = (pos_flat[1536 * nt + Phi] * w).sum(-1)
    wmap = 2 * (np.arange(24) % 12) + np.arange(24) // 12     # p(w)
    posx2 = xe4_pos[:, wmap, :].transpose(2, 0, 1)            # [d, t, w]
    c["posx2"] = posx2.reshape(D, NT * 24)
    W2p = np.zeros((205, 384), np.float32)
    for nt in range(NT):
        for p_ in range(P):
            j = nt * 24 + p_
            for k in range(P):
                m = 12 * nt + k
                W2p[min(m, 191), j] += patch_w[p_, 0, k]   # replicate-pad fold
            W2p[204, j] = patch_b[p_]
    c["w2pa"] = W2p[0:128]
    c["w2pb"] = np.concatenate([W2p[128:192], W2p[204:205]], 0)
    A = Wi.transpose(0, 3, 4, 2, 1).reshape(LAYERS, 9, D, D)                  # [l,tap,i,o]
    c["wiT"] = A.transpose(2, 0, 1, 3).reshape(D, LAYERS * 9 * D)
    Wf = Wi[:, :, :, ::-1, ::-1]
    Bt = Wf.transpose(0, 3, 4, 1, 2).reshape(LAYERS, 9, D, D)                 # [l,tap,o,i]
    c["w2T"] = Bt.transpose(2, 0, 1, 3).reshape(D, LAYERS * 9 * D)
    Ct = pconv_w.transpose(0, 3, 2, 1) / 24.0
    c["pcvT"] = Ct.transpose(2, 0, 1, 3).reshape(D, LAYERS * 3 * D)
    c["awT"] = (aconv_w.transpose(2, 0, 1) / 16.0).reshape(D, LAYERS * D)
    c["bns"] = (bn_g / np.sqrt(1.0 + BN_EPS)).T
    c["fc1e"] = fc1_w.T.reshape(D, 3 * PRED)
    c["fc1b"] = fc1_b[None, :]
    c["fc2T"] = fc2_w.T
    c["fc2b"] = fc2_b[None, :]
    c["bnb"] = (pconv_b * (bn_g / np.sqrt(1.0 + BN_EPS)) + bn_b).T

    # ---- composed all-sample stats kernels (no collectives) ----
    patch_w2 = patch_w[:, 0, :]
    pe_t = pe[(12 * np.arange(NT)[:, None] + np.arange(12)[None, :]).reshape(-1)].reshape(NT, 12, D)
    dens0, bias0 = _density(range(P), patch_w2, patch_b)
    G0, C0 = _compose_G(dens0, bias0, token_w, pe_t)
    dxe = {p: _density([p], patch_w2, patch_b) for p in (0, 2, 21, 23)}
    cxd = [
        (dens0 - dxe[21][0] - dxe[23][0], bias0 - dxe[21][1] - dxe[23][1]),
        (dens0 - dxe[0][0] - dxe[23][0], bias0 - dxe[0][1] - dxe[23][1]),
        (dens0 - dxe[0][0] - dxe[2][0], bias0 - dxe[0][1] - dxe[2][1]),
    ]
    G2 = np.zeros((D, CIN, 38), np.float32)
    C2 = np.zeros((D, 14), np.float32)
    for dq in range(3):
        Gcx, Ccx = _compose_G(cxd[dq][0], cxd[dq][1], token_w, pe_t)
        for dp in range(3):
            Wt = Wi[0][:, :, dp, dq]
            G2[:, :, 12 * dp:12 * dp + 14] += np.einsum("oi,icj->ocj", Wt, Gcx)
            C2 += np.einsum("oi,it->ot", Wt, Ccx[:, dp:dp + 14])
    G0T = G0.reshape(D, CIN * 14).T
    c["g0T"] = np.stack([G0T[98 * i:98 * (i + 1)] for i in range(3)], 1).reshape(98, 3 * 128)
    G2T = G2.reshape(D, CIN * 38).T
    c["g2T"] = np.stack([G2T[114 * i:114 * (i + 1)] for i in range(7)], 1).reshape(114, 7 * 128)
    c["cpp0r"] = np.broadcast_to(C0[:, None, :], (D, B, NT)).reshape(D, -1)
    c["c2r"] = np.broadcast_to(C2[:, None, :], (D, B, 14)).reshape(D, -1)
    wrfn = Wi[0][:, :, ::-1, :].sum(-1)                                       # [o, i, dp]
    c["wrf"] = wrfn.transpose(0, 2, 1).reshape(D, 3 * D)
    return c


def _build():
    nc = bacc.Bacc("TRN2", target_bir_lowering=False, debug=False, num_devices=N_CORES)

    xtp = nc.declare_dram_parameter("xtp", [CIN, BPC * 194], BF16, isOutput=False)
    pk_params = {}
    for pname, (parts, dt, pieces) in PACKS.items():
        total = sum(w for _, _, w in pieces)
        pk_params[pname] = nc.declare_dram_parameter(
            pname, [parts, total], F32 if dt == "f32" else BF16, isOutput=False)
    out = nc.declare_dram_parameter("out", [BPC, PRED, COUT], F32, isOutput=True)

    RELU = mybir.ActivationFunctionType.Relu
    SQUARE = mybir.ActivationFunctionType.Square
    ADD = mybir.AluOpType.add
    AX = mybir.AxisListType.X

    with tile.TileContext(nc) as tc:
        with tc.tile_pool(name="w", bufs=1) as wp, \
             tc.tile_pool(name="act", bufs=2) as ap, \
             tc.tile_pool(name="x2p", bufs=6) as xp, \
             tc.tile_pool(name="ps", bufs=1, space="PSUM") as pp:

            # --- packed param loads, triggers spread across engine queues ---
            pieces = {}
            dma_eng = {"pkt": nc.sync, "pkm": nc.scalar}
            xt_sb = wp.tile([CIN, BPC * 194], BF16, tag="xt")
            nc.sync.dma_start(out=xt_sb[:], in_=xtp[:, :])
            for pname in ("pkt", "pkm", "pkc", "pkw"):
                # pkc/pkw triggers are issued after the embedding block
                parts, dt, plist = PACKS[pname]
                total = sum(w for _, _, w in plist)
                t = wp.tile([parts, total], F32 if dt == "f32" else BF16, tag=pname)
                if pname not in ("pkw", "pkc"):
                    dma_eng[pname].dma_start(out=t[:], in_=pk_params[pname][:, :])
                o = 0
                for nm, pparts, w_ in plist:
                    pieces[nm] = (t, pparts, o)
                    o += w_
            pkw_tile = pieces["wiT"][0]
            pkc_tile = pieces["g0T"][0]
            # WAW corner-writes hold the big packs off the DMA rings until the
            # urgent embedding loads (xt/pkt/pkm) have drained
            nc.scalar.copy(out=pkc_tile[0:1, 0:1], in_=xt_sb[0:1, 0:1])
            nc.scalar.copy(out=pkw_tile[0:1, 0:1], in_=xt_sb[0:1, 0:1])
            nc.scalar.dma_start(out=pkc_tile[:], in_=pk_params["pkc"][:, :])
            nc.scalar.dma_start(out=pkw_tile[:], in_=pk_params["pkw"][:, :])

            def pv(nm, a, b_):
                t, pparts, o = pieces[nm]
                return t[0:pparts, o + a:o + b_]

            bnsb = wp.tile([D, 4], F32, tag="bnsb")
            ones_sb = wp.tile([1, D], BF16, tag="ones")
            nc.vector.memset(ones_sb[:], 1.0)
            ryspad = wp.tile([D, B, 18], BF16, tag="ryspad")
            nc.vector.memset(ryspad[:], 0.0)
            y0pads = {}
            for l in range(LAYERS):
                for n in range(BPC):
                    y0p = ap.tile([D, 18, 26], BF16, tag="y0p", bufs=4)
                    nc.vector.memset(y0p[:], 0.0)   # borders only matter
                    y0pads[(l, n)] = y0p

            # ---------------- embedding (own 2 samples) ----------------
            x2 = []
            for n in range(BPC):
                embs = []
                for tok in ("tokA", "tokR"):
                    e_ps = pp.tile([D, L], F32, tag="ps", bufs=6)
                    for k in range(3):
                        nc.tensor.matmul(e_ps[:], lhsT=pv(tok, k * D, (k + 1) * D),
                                         rhs=xt_sb[:, 194 * n + k:194 * n + k + L],
                                         start=(k == 0), stop=(k == 2))
                    embs.append(e_ps)
                eT = embs[0][:].rearrange("p (s c) -> p s c", c=3)
                eR = embs[1][:].rearrange("p (s c) -> p s c", c=3)
                xeA = ap.tile([D, 64, 2], BF16, tag="xeA")
                xeB = ap.tile([65, 64, 2], BF16, tag="xeB")
                nc.vector.tensor_copy(out=xeA[:, :, 0], in_=eT[:, :, 0])
                nc.vector.tensor_copy(out=xeA[0:64, :, 1], in_=eR[0:64, :, 1])
                nc.vector.tensor_copy(out=xeA[64:128, :, 1], in_=eR[64:128, :, 2])
                nc.scalar.copy(out=xeB[0:64, :, 0], in_=eT[0:64, :, 1])
                nc.scalar.copy(out=xeB[0:64, :, 1], in_=eR[0:64, :, 2])
                nc.vector.memset(xeB[64:65, :, :], 1.0)
                pcs = []
                for e in range(3):
                    pc_ps = pp.tile([D, NT, 8], F32, tag="ps", bufs=6)
                    nc.tensor.matmul(pc_ps[:], lhsT=pv("w2pa", 128 * e, 128 * (e + 1)),
                                     rhs=xeA[:], start=True, stop=False)
                    nc.tensor.matmul(pc_ps[:], lhsT=pv("w2pb", 128 * e, 128 * (e + 1)),
                                     rhs=xeB[:], start=False, stop=True)
                    pcs.append(pc_ps)
                # psum -> x2 permutation as 6 strided AP copies:
                # x2 w = 3k+r <-> pcs[e] q = 2k2+r2 with p(w) = 2*(w%12) + w//12
                x2n = xp.tile([D, NT, 24], F32, tag="x2")
                x2v = x2n[:].rearrange("p t (k r) -> p t k r", r=3)
                px2 = pv("posx2", 0, 384).rearrange("p (t k r) -> p t k r", k=8, r=3)
                plan = [(0, 0, 0, 0), (0, 1, 1, 1),
                        (1, 2, 0, 1), (1, 0, 1, 0),
                        (2, 1, 0, 0), (2, 2, 1, 1)]
                for i, (e, r, half, r2) in enumerate(plan):
                    pcv = pcs[e][:].rearrange("p t (k q) -> p t k q", q=2)
                    eng = nc.vector.tensor_add
                    eng(out=x2v[:, :, 4 * half:4 * (half + 1), r],
                        in0=pcv[:, :, :, r2],
                        in1=px2[:, :, 4 * half:4 * (half + 1), r])
                x2.append(x2n)

            nc.vector.tensor_copy(out=bnsb[:], in_=pv("bns", 0, 4))

            # ------- stats pipeline pieces (interleaved with conv blocks) -------
            def ppc_pool(l, ppsb):
                ppc_ps = pp.tile([D, B, NT], F32, tag="st", bufs=2)
                for k in range(3):
                    o = (l * 3 + k) * D
                    if k == 0:
                        nc.tensor.matmul(ppc_ps[:, :, 1:NT], lhsT=pv("pcvT", o, o + D),
                                         rhs=ppsb[:, :, 0:NT - 1], start=True, stop=False)
                    elif k == 1:
                        nc.tensor.matmul(ppc_ps[:], lhsT=pv("pcvT", o, o + D),
                                         rhs=ppsb[:], start=False, stop=False)
                    else:
                        nc.tensor.matmul(ppc_ps[:, :, 0:NT - 1], lhsT=pv("pcvT", o, o + D),
                                         rhs=ppsb[:, :, 1:NT], start=False, stop=True)
                ppc_sb = ap.tile([D, B, NT], F32, tag="ppc_sb", bufs=2)
                nc.scalar.activation(out=ppc_sb[:], in_=ppc_ps[:], func=RELU,
                                     bias=bnsb[:, 2 + l:3 + l], scale=bnsb[:, l:l + 1])
                pooled = ap.tile([D, B], BF16, tag="pooled", bufs=2)
                with nc.allow_low_precision(reason="16-term pooled mean, tol 2e-2"):
                    nc.vector.tensor_reduce(out=pooled[:], in_=ppc_sb[:], axis=AX, op=ADD)
                return pooled

            def alpha_sq(l, pooled):
                al_ps = pp.tile([D, B], F32, tag="st", bufs=2)
                nc.tensor.matmul(al_ps[:], lhsT=pv("awT", l * D, (l + 1) * D),
                                 rhs=pooled[:], start=True, stop=True)
                asq = ap.tile([D, B], F32, tag="asq", bufs=2)
                s_sb = ap.tile([D, 1], F32, tag="s", bufs=2)
                nc.scalar.activation(out=asq[:], in_=al_ps[:], func=SQUARE,
                                     bias=1.0, scale=1.0, accum_out=s_sb[:])
                return s_sb

            def conv9(out_ps, wname, base, rhs_tile, windows):
                for dp in range(3):
                    for dq in range(3):
                        tap = 3 * dp + dq
                        o = base + tap * D
                        nc.tensor.matmul(out_ps[:], lhsT=pv(wname, o, o + D),
                                         rhs=rhs_tile[:, dp:dp + windows[0],
                                                      dq:dq + windows[1]],
                                         start=(tap == 0), stop=(tap == 8))

            # stats phase 1: pp0, ry0, ppc0 -> pooled0
            pp0_ps = pp.tile([D, B, NT], F32, tag="st", bufs=2)
            for cc in range(3):
                nc.tensor.matmul(pp0_ps[:], lhsT=pv("g0T", 128 * cc, 128 * (cc + 1)),
                                 rhs=pv("xg", 256 * cc, 256 * (cc + 1)).rearrange(
                                     "p (b t) -> p b t", t=NT),
                                 start=(cc == 0), stop=(cc == 2))
            pp0_sb = wp.tile([D, B, NT], BF16, tag="pp0")
            nc.vector.tensor_add(out=pp0_sb[:], in0=pp0_ps[:],
                                 in1=pv("cpp0r", 0, 256).rearrange("p (b t) -> p b t", t=NT))
            ry0_ps = pp.tile([D, B, 14], F32, tag="st", bufs=2)
            for cc in range(7):
                nc.tensor.matmul(ry0_ps[:], lhsT=pv("g2T", 128 * cc, 128 * (cc + 1)),
                                 rhs=pv("xg2", 224 * cc, 224 * (cc + 1)).rearrange(
                                     "p (b r) -> p b r", r=14),
                                 start=(cc == 0), stop=(cc == 6))
            ry0_sb = ap.tile([D, B, 14], F32, tag="ry0")
            nc.vector.tensor_add(out=ry0_sb[:], in0=ry0_ps[:],
                                 in1=pv("c2r", 0, 224).rearrange("p (b r) -> p b r", r=14))
            pooled0 = ppc_pool(0, pp0_sb)

            # conv l=0 part A: casts + Y0
            x2b = []
            for n in range(BPC):
                x2bn = ap.tile([D, NT, 24], BF16, tag="x2b", bufs=2)
                (nc.vector.tensor_copy if n == 0 else
                 (lambda out, in_: nc.scalar.copy(out=out, in_=in_)))(
                    out=x2bn[:], in_=x2[n][:])
                x2b.append(x2bn)
            y0ps = []
            for n in range(BPC):
                y0_ps = pp.tile([D, 14, 22], F32, tag="ps", bufs=6)
                conv9(y0_ps, "wiT", 0, x2b[n], (14, 22))
                y0ps.append(y0_ps)

            # stats phase 2: s0, rsz -> pp1 -> pooled1
            s0_sb = alpha_sq(0, pooled0)
            nc.vector.tensor_scalar_mul(out=ryspad[:, :, 2:16], in0=ry0_sb[:],
                                        scalar1=s0_sb[:])
            rsz_ps = pp.tile([D, B, NT], F32, tag="st", bufs=2)
            for dp in range(3):
                nc.tensor.matmul(rsz_ps[:], lhsT=pv("wrf", dp * D, (dp + 1) * D),
                                 rhs=ryspad[:, :, dp:dp + 16],
                                 start=(dp == 0), stop=(dp == 2))
            pp1_sb = wp.tile([D, B, NT], BF16, tag="pp1")
            nc.vector.tensor_add(out=pp1_sb[:], in0=rsz_ps[:], in1=pp0_sb[:])
            pooled1 = ppc_pool(1, pp1_sb)

            # conv l=0 part B: scale + z + residual
            for n in range(BPC):
                y0p = y0pads[(0, n)]
                nc.vector.tensor_scalar_mul(out=y0p[:, 2:16, 2:24], in0=y0ps[n][:],
                                            scalar1=s0_sb[:])
                z_ps = pp.tile([D, NT, 24], F32, tag="ps", bufs=6)
                conv9(z_ps, "w2T", 0, y0p, (16, 24))
                x2n = xp.tile([D, NT, 24], F32, tag="x2")
                nc.vector.tensor_add(out=x2n[:], in0=z_ps[:], in1=x2[n][:])
                x2[n] = x2n

            s1_sb = alpha_sq(1, pooled1)

            # conv l=1
            x2b = []
            for n in range(BPC):
                x2bn = ap.tile([D, NT, 24], BF16, tag="x2b", bufs=2)
                (nc.vector.tensor_copy if n == 0 else
                 (lambda out, in_: nc.scalar.copy(out=out, in_=in_)))(
                    out=x2bn[:], in_=x2[n][:])
                x2b.append(x2bn)
            y0ps = []
            for n in range(BPC):
                y0_ps = pp.tile([D, 14, 22], F32, tag="ps", bufs=6)
                conv9(y0_ps, "wiT", 9 * D, x2b[n], (14, 22))
                y0ps.append(y0_ps)
            x2fin = []
            for n in range(BPC):
                y0p = y0pads[(1, n)]
                nc.vector.tensor_scalar_mul(out=y0p[:, 2:16, 2:24], in0=y0ps[n][:],
                                            scalar1=s1_sb[:])
                z_ps = pp.tile([D, NT, 24], F32, tag="ps", bufs=6)
                conv9(z_ps, "w2T", 9 * D, y0p, (16, 24))
                x2h = ap.tile([D, NT, 24], BF16, tag="x2h", bufs=2)
                with nc.allow_low_precision(reason="head input, tol 2e-2"):
                    nc.vector.tensor_add(out=x2h[:], in0=z_ps[:], in1=x2[n][:])
                x2fin.append(x2h)

            # ---------------- heads ----------------
            o_all = ap.tile([PRED, BPC, COUT], F32, tag="oall")
            for n in range(BPC):
                x2f = x2fin[n][:].rearrange("p a b -> p (a b)")
                y1_ps = pp.tile([D, PRED], F32, tag="ps", bufs=6)
                for e in range(3):
                    nc.tensor.matmul(y1_ps[:], lhsT=x2f[:, 128 * e:128 * (e + 1)],
                                     rhs=pv("fc1e", PRED * e, PRED * (e + 1)),
                                     start=(e == 0), stop=False)
                nc.tensor.matmul(y1_ps[:], lhsT=ones_sb[:], rhs=pv("fc1b", 0, PRED),
                                 start=False, stop=True)
                y1_sb = ap.tile([D, PRED], BF16, tag="y1sb")
                nc.scalar.copy(out=y1_sb[:], in_=y1_ps[:])
                o_ps = pp.tile([PRED, COUT], F32, tag="ps", bufs=6)
                nc.tensor.matmul(o_ps[:], lhsT=y1_sb[:], rhs=pv("fc2T", 0, COUT),
                                 start=True, stop=False)
                nc.tensor.matmul(o_ps[:], lhsT=ones_sb[:, 0:PRED], rhs=pv("fc2b", 0, COUT),
                                 start=False, stop=True)
                (nc.vector.tensor_copy if n == 0 else
                 (lambda out, in_: nc.scalar.copy(out=out, in_=in_)))(
                    out=o_all[:, n, :], in_=o_ps[:])
            nc.sync.dma_start(out=out[:, :, :].rearrange("n p c -> p n c"),
                              in_=o_all[:])

    nc.finalize()
    return nc


def kernel(**inputs):
    global LAST_RESULT
    import ml_dtypes
    inputs = {k: np.ascontiguousarray(np.asarray(v, np.float32)) for k, v in inputs.items()}
    if "nc" not in _CACHE:
        _CACHE["nc"] = _build()
    nc = _CACHE["nc"]
    c = _prep_consts(
        inputs["token_w"], inputs["patch_w"], inputs["patch_b"], inputs["Wi"],
        inputs["pconv_w"], inputs["pconv_b"], inputs["bn_g"], inputs["bn_b"],
        inputs["aconv_w"], inputs["fc1_w"], inputs["fc1_b"], inputs["fc2_w"],
        inputs["fc2_b"])
    xtp_full = np.pad(inputs["x"].transpose(0, 2, 1), ((0, 0), (0, 0), (1, 1)),
                      mode="wrap").astype(np.float32)
    # im2col gathers of x for the stats path (identical on all cores)
    xG = np.empty((CIN, 14, B, NT), np.float32)
    for jp in range(14):
        xG[:, jp] = xtp_full[:, :, jp::12][:, :, :NT].transpose(1, 0, 2)
    xG = xG.reshape(CIN * 14, B * NT)
    c["xg"] = np.stack([xG[98 * i:98 * (i + 1)] for i in range(3)], 1).reshape(98, 3 * 256)
    xG2 = np.empty((CIN, 38, B, 14), np.float32)
    for jp in range(38):
        xG2[:, jp] = xtp_full[:, :, jp::12][:, :, :14].transpose(1, 0, 2)
    xG2 = xG2.reshape(CIN * 38, B * 14)
    c["xg2"] = np.stack([xG2[114 * i:114 * (i + 1)] for i in range(7)], 1).reshape(114, 7 * 224)

    base = {}
    for pname, (parts, dt, plist) in PACKS.items():
        cols = []
        for nm, pparts, w_ in plist:
            a = np.zeros((parts, w_), np.float32)
            a[:pparts] = np.asarray(c[nm], np.float32).reshape(pparts, w_)
            cols.append(a)
        arr = np.concatenate(cols, axis=1)
        base[pname] = np.ascontiguousarray(
            arr.astype(ml_dtypes.bfloat16 if dt == "bf16" else np.float32))
    in_maps = []
    for core in range(N_CORES):
        m = dict(base)
        xt = np.concatenate([xtp_full[BPC * core + n] for n in range(BPC)], axis=1)
        m["xtp"] = np.ascontiguousarray(xt.astype(ml_dtypes.bfloat16))
        in_maps.append(m)
    import os
    res = run_bass_kernel_spmd(nc, in_maps, core_ids=list(range(N_CORES)),
                               trace=bool(os.environ.get("BASS_TRACE")))
    LAST_RESULT = res
    return np.concatenate([res.results[cid]["out"] for cid in range(N_CORES)], axis=0)


# revision 20
# speedup vs baseline: 1.0572x; 1.0057x over previous
"""Self-contained Trainium2 Bass kernel for nn_Model_16801912062040 (dense_cnn).

Sharding: batch-parallel, 2 samples per core across 8 cores, ZERO collectives.
The dynamic conv collapses algebraically: y[n,(m,o)] = alpha[m,o]*(x2[n] (x) Wi[o])
and the conv_transpose contraction over (m,o) reduces to
    z[n,i] = sum_o s_o * (Y0[n,o] (x)_full flip(Wi[o,i])),  s_o = sum_m alpha[m,o]^2
so only the per-channel scalar s (128 floats per layer) couples samples.

Every core computes s for BOTH layers locally from the full x: all pooled
statistics are linear in x up to the alpha nonlinearity, and the raster reshape
(B,L,D)->(B*D,1,L) has an exact shift structure in nt (1536 flat elements = 12
l-steps), so pp0 and the Y0 row-sums (ry0) for all 16 samples are stride-12
convs of x with host-composed kernels (G0: 14 taps; G2: 38 taps folding Wi and
the windowed-column-sum edge corrections); row-sums of z then come from 3
matmuls against row-flipped row-summed Wi (wrf), giving pp1 and s1. Each core
is fully independent: no barriers, no skew amplification.

Perf notes:
- fp32 matmuls run ~4x slower per column than bf16 -> every matmul input is
  bf16 (PSUM accumulation stays f32).
- Each dma_start costs ~700ns serialized on its engine's sequencer, so params
  are packed into 6 loads spread across different engine queues.
- The stats pipeline is interleaved between conv blocks so its cross-engine
  latency hides behind matmul streams instead of stalling the PE queue.
- The psum->x2 gather is 6 strided AP copies instead of 24 per sample.
"""
import math
import numpy as np

import concourse.bass as bass
import concourse.tile as tile
from concourse import bacc, mybir
from concourse.bass_utils import run_bass_kernel_spmd

N_CORES = 8
BPC = 2            # samples per core
B, L, CIN, D = 16, 192, 21, 128
P, S, NT, PRED, COUT = 24, 12, 16, 96, 21
LAYERS = 2
BN_EPS = 1e-5
F32 = mybir.dt.float32
BF16 = mybir.dt.bfloat16

# pack -> (partitions, dtype, [(piece, piece_partitions, cols), ...])
PACKS = {
    "pkm": (128, "bf16", [("posx2", 128, 384),
                          ("w2pa", 128, 384), ("w2pb", 65, 384)]),
    "pkw1": (128, "bf16", [("wiT0", 128, 1152), ("w2T0", 128, 1152)]),
    "pkw2": (128, "bf16", [("wiT1", 128, 1152), ("w2T1", 128, 1152)]),
    "pkt": (21, "bf16", [("tokA", 21, 384), ("tokR", 21, 384)]),
    "pkc": (128, "bf16", [("bns", 128, 2), ("bnb", 128, 2),
                          ("cpp0r", 128, 256), ("c2r", 128, 224),
                          ("pcvT", 128, 768), ("awT", 128, 256),
                          ("wrf", 128, 384), ("fc1e", 128, 288),
                          ("fc2T", 128, 21), ("g0T", 98, 384), ("xg", 98, 768),
                          ("g2T", 114, 896), ("xg2", 114, 1568),
                          ("fc1b", 1, 96), ("fc2b", 1, 21)]),
}

_CACHE = {}
LAST_RESULT = None


def _pos_embed():
    pos = np.arange(L, dtype=np.float32)[:, None]
    div = np.exp(np.arange(0, D, 2, dtype=np.float32) * (-math.log(10000.0) / D))
    pe = np.zeros((L, D), np.float32)
    pe[:, 0::2] = np.sin(pos * div)
    pe[:, 1::2] = np.cos(pos * div)
    return pe


def _slice_map(p, patch_w2, patch_b):
    """Phi/weight/bias map for xe4[:, nt, p, d_t] in E-flat space (nt-shift-free)."""
    d_t = np.arange(D)
    g = p // 3
    inner = (p % 3) * 128 + d_t
    nts = inner // 24
    ps = inner % 24
    k = np.arange(P)
    j = np.minimum(12 * nts[:, None] + k[None, :], 191)   # edge-pad fold
    Phi = 192 * g + j
    return Phi, patch_w2[ps], patch_b[ps]


def _density(ps_list, patch_w2, patch_b):
    d_t = np.arange(D)
    dens = np.zeros((D, 12, 128), np.float32)
    bias = np.zeros(D, np.float32)
    for p in ps_list:
        Phi, w, b = _slice_map(p, patch_w2, patch_b)
        for k in range(P):
            np.add.at(dens, (d_t, Phi[:, k] // 128, Phi[:, k] % 128), w[:, k])
        bias += b
    return dens, bias


def _compose_G(dens, bias, token_w, pe_t):
    G = np.zeros((D, CIN, 14), np.float32)
    for k2 in range(3):
        G[:, :, k2:k2 + 12] += np.einsum("dlm,mc->dcl", dens, token_w[:, :, k2])
    C = np.einsum("dlm,tlm->dt", dens, pe_t) + bias[:, None]
    return G, C


def _prep_consts(token_w, patch_w, patch_b, Wi, pconv_w, pconv_b, bn_g, bn_b,
                 aconv_w, fc1_w, fc1_b, fc2_w, fc2_b):
    c = {}
    c["tokA"] = token_w.transpose(1, 2, 0).reshape(CIN, -1)
    c["tokR"] = np.roll(token_w, -64, 0).transpose(1, 2, 0).reshape(CIN, -1)
    pe = _pos_embed()
    pos_flat = pe.reshape(L * D)
    xe4_pos = np.zeros((NT, P, D), np.float32)
    for p_ in range(P):
        Phi, w, _ = _slice_map(p_, patch_w[:, 0, :], patch_b)
        for nt in range(NT):
            xe4_pos[nt, p_] = (pos_flat[1536 * nt + Phi] * w).sum(-1)
    wmap = 2 * (np.arange(24) % 12) + np.arange(24) // 12     # p(w)
    posx2 = xe4_pos[:, wmap, :].transpose(2, 0, 1)            # [d, t, w]
    c["posx2"] = posx2.reshape(D, NT * 24)
    W2p = np.zeros((205, 384), np.float32)
    for nt in range(NT):
        for p_ in range(P):
            j = nt * 24 + p_
            for k in range(P):
                m = 12 * nt + k
                W2p[min(m, 191), j] += patch_w[p_, 0, k]   # replicate-pad fold
            W2p[204, j] = patch_b[p_]
    c["w2pa"] = W2p[0:128]
    c["w2pb"] = np.concatenate([W2p[128:192], W2p[204:205]], 0)
    A = Wi.transpose(0, 3, 4, 2, 1).reshape(LAYERS, 9, D, D)                  # [l,tap,i,o]
    wiT = A.transpose(2, 0, 1, 3).reshape(D, LAYERS * 9 * D)
    c["wiT0"], c["wiT1"] = wiT[:, :9 * D], wiT[:, 9 * D:]
    Wf = Wi[:, :, :, ::-1, ::-1]
    Bt = Wf.transpose(0, 3, 4, 1, 2).reshape(LAYERS, 9, D, D)                 # [l,tap,o,i]
    w2T = Bt.transpose(2, 0, 1, 3).reshape(D, LAYERS * 9 * D)
    c["w2T0"], c["w2T1"] = w2T[:, :9 * D], w2T[:, 9 * D:]
    Ct = pconv_w.transpose(0, 3, 2, 1) / 24.0
    c["pcvT"] = Ct.transpose(2, 0, 1, 3).reshape(D, LAYERS * 3 * D)
    c["awT"] = (aconv_w.transpose(2, 0, 1) / 16.0).reshape(D, LAYERS * D)
    c["bns"] = (bn_g / np.sqrt(1.0 + BN_EPS)).T
    c["fc1e"] = fc1_w.T.reshape(D, 3 * PRED)
    c["fc1b"] = fc1_b[None, :]
    c["fc2T"] = fc2_w.T
    c["fc2b"] = fc2_b[None, :]
    c["bnb"] = (pconv_b * (bn_g / np.sqrt(1.0 + BN_EPS)) + bn_b).T

    # ---- composed all-sample stats kernels (no collectives) ----
    patch_w2 = patch_w[:, 0, :]
    pe_t = pe[(12 * np.arange(NT)[:, None] + np.arange(12)[None, :]).reshape(-1)].reshape(NT, 12, D)
    dens0, bias0 = _density(range(P), patch_w2, patch_b)
    G0, C0 = _compose_G(dens0, bias0, token_w, pe_t)
    dxe = {p: _density([p], patch_w2, patch_b) for p in (0, 2, 21, 23)}
    cxd = [
        (dens0 - dxe[21][0] - dxe[23][0], bias0 - dxe[21][1] - dxe[23][1]),
        (dens0 - dxe[0][0] - dxe[23][0], bias0 - dxe[0][1] - dxe[23][1]),
        (dens0 - dxe[0][0] - dxe[2][0], bias0 - dxe[0][1] - dxe[2][1]),
    ]
    G2 = np.zeros((D, CIN, 38), np.float32)
    C2 = np.zeros((D, 14), np.float32)
    for dq in range(3):
        Gcx, Ccx = _compose_G(cxd[dq][0], cxd[dq][1], token_w, pe_t)
        for dp in range(3):
            Wt = Wi[0][:, :, dp, dq]
            G2[:, :, 12 * dp:12 * dp + 14] += np.einsum("oi,icj->ocj", Wt, Gcx)
            C2 += np.einsum("oi,it->ot", Wt, Ccx[:, dp:dp + 14])
    G0T = G0.reshape(D, CIN * 14).T
    c["g0T"] = np.stack([G0T[98 * i:98 * (i + 1)] for i in range(3)], 1).reshape(98, 3 * 128)
    G2T = G2.reshape(D, CIN * 38).T
    c["g2T"] = np.stack([G2T[114 * i:114 * (i + 1)] for i in range(7)], 1).reshape(114, 7 * 128)
    c["cpp0r"] = np.broadcast_to(C0[:, None, :], (D, B, NT)).reshape(D, -1)
    c["c2r"] = np.broadcast_to(C2[:, None, :], (D, B, 14)).reshape(D, -1)
    wrfn = Wi[0][:, :, ::-1, :].sum(-1)                                       # [o, i, dp]
    c["wrf"] = wrfn.transpose(0, 2, 1).reshape(D, 3 * D)
    return c


def _build():
    nc = bacc.Bacc("TRN2", target_bir_lowering=False, debug=False, num_devices=N_CORES)

    xtp = nc.declare_dram_parameter("xtp", [CIN, BPC * 194], BF16, isOutput=False)
    pk_params = {}
    for pname, (parts, dt, pieces) in PACKS.items():
        total = sum(w for _, _, w in pieces)
        pk_params[pname] = nc.declare_dram_parameter(
            pname, [parts, total], F32 if dt == "f32" else BF16, isOutput=False)
    out = nc.declare_dram_parameter("out", [BPC, PRED, COUT], F32, isOutput=True)

    RELU = mybir.ActivationFunctionType.Relu
    SQUARE = mybir.ActivationFunctionType.Square
    ADD = mybir.AluOpType.add
    AX = mybir.AxisListType.X

    with tile.TileContext(nc) as tc:
        with tc.tile_pool(name="w", bufs=1) as wp, \
             tc.tile_pool(name="act", bufs=2) as ap, \
             tc.tile_pool(name="x2p", bufs=6) as xp, \
             tc.tile_pool(name="ps", bufs=1, space="PSUM") as pp:

            # --- packed param loads, triggers spread across engine queues ---
            pieces = {}
            dma_eng = {"pkt": nc.sync, "pkm": nc.scalar, "pkw1": nc.scalar}
            xt_sb = wp.tile([CIN, BPC * 194], BF16, tag="xt")
            nc.sync.dma_start(out=xt_sb[:], in_=xtp[:, :])
            for pname in ("pkt", "pkm", "pkw1", "pkc", "pkw2"):
                parts, dt, plist = PACKS[pname]
                total = sum(w for _, _, w in plist)
                t = wp.tile([parts, total], F32 if dt == "f32" else BF16, tag=pname)
                if pname not in ("pkw2", "pkc"):
                    dma_eng[pname].dma_start(out=t[:], in_=pk_params[pname][:, :])
                o = 0
                for nm, pparts, w_ in plist:
                    pieces[nm] = (t, pparts, o)
                    o += w_
            pkw2_tile = pieces["wiT1"][0]
            pkc_tile = pieces["g0T"][0]
            # WAW corner-writes hold the wave-2 packs off the DMA rings until
            # the urgent embedding loads (xt/pkt/pkm/pkw1) have drained
            nc.scalar.copy(out=pkc_tile[0:1, 0:1], in_=xt_sb[0:1, 0:1])
            nc.vector.tensor_copy(out=pkw2_tile[0:1, 0:1], in_=xt_sb[0:1, 0:1])
            nc.scalar.dma_start(out=pkc_tile[:], in_=pk_params["pkc"][:, :])
            nc.sync.dma_start(out=pkw2_tile[:], in_=pk_params["pkw2"][:, :])

            def pv(nm, a, b_):
                t, pparts, o = pieces[nm]
                return t[0:pparts, o + a:o + b_]

            bnsb = wp.tile([D, 4], F32, tag="bnsb")
            ones_sb = wp.tile([1, D], BF16, tag="ones")
            nc.vector.memset(ones_sb[:], 1.0)
            ryspad = wp.tile([D, B, 18], BF16, tag="ryspad")
            nc.vector.memset(ryspad[:], 0.0)
            y0pads = {}
            for l in range(LAYERS):
                for n in range(BPC):
                    y0p = ap.tile([D, 18, 26], BF16, tag="y0p", bufs=4)
                    nc.vector.memset(y0p[:], 0.0)   # borders only matter
                    y0pads[(l, n)] = y0p

            # ---------------- embedding (own 2 samples) ----------------
            x2 = []
            for n in range(BPC):
                embs = []
                for tok in ("tokA", "tokR"):
                    e_ps = pp.tile([D, L], F32, tag="ps", bufs=6)
                    for k in range(3):
                        nc.tensor.matmul(e_ps[:], lhsT=pv(tok, k * D, (k + 1) * D),
                                         rhs=xt_sb[:, 194 * n + k:194 * n + k + L],
                                         start=(k == 0), stop=(k == 2))
                    embs.append(e_ps)
                eT = embs[0][:].rearrange("p (s c) -> p s c", c=3)
                eR = embs[1][:].rearrange("p (s c) -> p s c", c=3)
                xeA = ap.tile([D, 64, 2], BF16, tag="xeA")
                xeB = ap.tile([65, 64, 2], BF16, tag="xeB")
                nc.vector.tensor_copy(out=xeA[:, :, 0], in_=eT[:, :, 0])
                nc.vector.tensor_copy(out=xeA[0:64, :, 1], in_=eR[0:64, :, 1])
                nc.vector.tensor_copy(out=xeA[64:128, :, 1], in_=eR[64:128, :, 2])
                nc.scalar.copy(out=xeB[0:64, :, 0], in_=eT[0:64, :, 1])
                nc.scalar.copy(out=xeB[0:64, :, 1], in_=eR[0:64, :, 2])
                nc.vector.memset(xeB[64:65, :, :], 1.0)
                pcs = []
                for e in range(3):
                    pc_ps = pp.tile([D, NT, 8], F32, tag="ps", bufs=6)
                    nc.tensor.matmul(pc_ps[:], lhsT=pv("w2pa", 128 * e, 128 * (e + 1)),
                                     rhs=xeA[:], start=True, stop=False)
                    nc.tensor.matmul(pc_ps[:], lhsT=pv("w2pb", 128 * e, 128 * (e + 1)),
                                     rhs=xeB[:], start=False, stop=True)
                    pcs.append(pc_ps)
                # psum -> x2 permutation as 6 strided AP copies:
                # x2 w = 3k+r <-> pcs[e] q = 2k2+r2 with p(w) = 2*(w%12) + w//12
                x2n = xp.tile([D, NT, 24], F32, tag="x2")
                x2v = x2n[:].rearrange("p t (k r) -> p t k r", r=3)
                px2 = pv("posx2", 0, 384).rearrange("p (t k r) -> p t k r", k=8, r=3)
                plan = [(0, 0, 0, 0), (0, 1, 1, 1),
                        (1, 2, 0, 1), (1, 0, 1, 0),
                        (2, 1, 0, 0), (2, 2, 1, 1)]
                for i, (e, r, half, r2) in enumerate(plan):
                    pcv = pcs[e][:].rearrange("p t (k q) -> p t k q", q=2)
                    eng = nc.vector.tensor_add
                    eng(out=x2v[:, :, 4 * half:4 * (half + 1), r],
                        in0=pcv[:, :, :, r2],
                        in1=px2[:, :, 4 * half:4 * (half + 1), r])
                x2.append(x2n)

            nc.vector.tensor_copy(out=bnsb[:], in_=pv("bns", 0, 4))

            # ------- stats pipeline pieces (interleaved with conv blocks) -------
            def ppc_pool(l, ppsb):
                ppc_ps = pp.tile([D, B, NT], F32, tag="st", bufs=2)
                for k in range(3):
                    o = (l * 3 + k) * D
                    if k == 0:
                        nc.tensor.matmul(ppc_ps[:, :, 1:NT], lhsT=pv("pcvT", o, o + D),
                                         rhs=ppsb[:, :, 0:NT - 1], start=True, stop=False)
                    elif k == 1:
                        nc.tensor.matmul(ppc_ps[:], lhsT=pv("pcvT", o, o + D),
                                         rhs=ppsb[:], start=False, stop=False)
                    else:
                        nc.tensor.matmul(ppc_ps[:, :, 0:NT - 1], lhsT=pv("pcvT", o, o + D),
                                         rhs=ppsb[:, :, 1:NT], start=False, stop=True)
                ppc_sb = ap.tile([D, B, NT], F32, tag="ppc_sb", bufs=2)
                nc.scalar.activation(out=ppc_sb[:], in_=ppc_ps[:], func=RELU,
                                     bias=bnsb[:, 2 + l:3 + l], scale=bnsb[:, l:l + 1])
                pooled = ap.tile([D, B], BF16, tag="pooled", bufs=2)
                with nc.allow_low_precision(reason="16-term pooled mean, tol 2e-2"):
                    nc.vector.tensor_reduce(out=pooled[:], in_=ppc_sb[:], axis=AX, op=ADD)
                return pooled

            def alpha_sq(l, pooled):
                al_ps = pp.tile([D, B], F32, tag="st", bufs=2)
                nc.tensor.matmul(al_ps[:], lhsT=pv("awT", l * D, (l + 1) * D),
                                 rhs=pooled[:], start=True, stop=True)
                asq = ap.tile([D, B], F32, tag="asq", bufs=2)
                s_sb = ap.tile([D, 1], F32, tag="s", bufs=2)
                nc.scalar.activation(out=asq[:], in_=al_ps[:], func=SQUARE,
                                     bias=1.0, scale=1.0, accum_out=s_sb[:])
                return s_sb

            def conv9(out_ps, wname, base, rhs_tile, windows):
                for dp in range(3):
                    for dq in range(3):
                        tap = 3 * dp + dq
                        o = tap * D
                        nc.tensor.matmul(out_ps[:], lhsT=pv(wname, o, o + D),
                                         rhs=rhs_tile[:, dp:dp + windows[0],
                                                      dq:dq + windows[1]],
                                         start=(tap == 0), stop=(tap == 8))

            # stats phase 1: pp0, ry0, ppc0 -> pooled0
            pp0_ps = pp.tile([D, B, NT], F32, tag="st", bufs=2)
            for cc in range(3):
                nc.tensor.matmul(pp0_ps[:], lhsT=pv("g0T", 128 * cc, 128 * (cc + 1)),
                                 rhs=pv("xg", 256 * cc, 256 * (cc + 1)).rearrange(
                                     "p (b t) -> p b t", t=NT),
                                 start=(cc == 0), stop=(cc == 2))
            pp0_sb = wp.tile([D, B, NT], BF16, tag="pp0")
            nc.vector.tensor_add(out=pp0_sb[:], in0=pp0_ps[:],
                                 in1=pv("cpp0r", 0, 256).rearrange("p (b t) -> p b t", t=NT))
            ry0_ps = pp.tile([D, B, 14], F32, tag="st", bufs=2)
            for cc in range(7):
                nc.tensor.matmul(ry0_ps[:], lhsT=pv("g2T", 128 * cc, 128 * (cc + 1)),
                                 rhs=pv("xg2", 224 * cc, 224 * (cc + 1)).rearrange(
                                     "p (b r) -> p b r", r=14),
                                 start=(cc == 0), stop=(cc == 6))
            ry0_sb = ap.tile([D, B, 14], F32, tag="ry0")
            nc.vector.tensor_add(out=ry0_sb[:], in0=ry0_ps[:],
                                 in1=pv("c2r", 0, 224).rearrange("p (b r) -> p b r", r=14))
            pooled0 = ppc_pool(0, pp0_sb)

            # conv l=0 part A: casts + Y0
            x2b = []
            for n in range(BPC):
                x2bn = ap.tile([D, NT, 24], BF16, tag="x2b", bufs=2)
                (nc.vector.tensor_copy if n == 0 else
                 (lambda out, in_: nc.scalar.copy(out=out, in_=in_)))(
                    out=x2bn[:], in_=x2[n][:])
                x2b.append(x2bn)
            y0ps = []
            for n in range(BPC):
                y0_ps = pp.tile([D, 14, 22], F32, tag="ps", bufs=6)
                conv9(y0_ps, "wiT0", 0, x2b[n], (14, 22))
                y0ps.append(y0_ps)

            # stats phase 2: s0, rsz -> pp1 -> pooled1
            s0_sb = alpha_sq(0, pooled0)
            nc.vector.tensor_scalar_mul(out=ryspad[:, :, 2:16], in0=ry0_sb[:],
                                        scalar1=s0_sb[:])
            rsz_ps = pp.tile([D, B, NT], F32, tag="st", bufs=2)
            for dp in range(3):
                nc.tensor.matmul(rsz_ps[:], lhsT=pv("wrf", dp * D, (dp + 1) * D),
                                 rhs=ryspad[:, :, dp:dp + 16],
                                 start=(dp == 0), stop=(dp == 2))
            pp1_sb = wp.tile([D, B, NT], BF16, tag="pp1")
            nc.vector.tensor_add(out=pp1_sb[:], in0=rsz_ps[:], in1=pp0_sb[:])
            pooled1 = ppc_pool(1, pp1_sb)

            # conv l=0 part B: scale + z + residual
            for n in range(BPC):
                y0p = y0pads[(0, n)]
                nc.vector.tensor_scalar_mul(out=y0p[:, 2:16, 2:24], in0=y0ps[n][:],
                                            scalar1=s0_sb[:])
                z_ps = pp.tile([D, NT, 24], F32, tag="ps", bufs=6)
                conv9(z_ps, "w2T0", 0, y0p, (16, 24))
                x2n = xp.tile([D, NT, 24], F32, tag="x2")
                nc.vector.tensor_add(out=x2n[:], in0=z_ps[:], in1=x2[n][:])
                x2[n] = x2n

            s1_sb = alpha_sq(1, pooled1)

            # conv l=1
            x2b = []
            for n in range(BPC):
                x2bn = ap.tile([D, NT, 24], BF16, tag="x2b", bufs=2)
                (nc.vector.tensor_copy if n == 0 else
                 (lambda out, in_: nc.scalar.copy(out=out, in_=in_)))(
                    out=x2bn[:], in_=x2[n][:])
                x2b.append(x2bn)
            y0ps = []
            for n in range(BPC):
                y0_ps = pp.tile([D, 14, 22], F32, tag="ps", bufs=6)
                conv9(y0_ps, "wiT1", 0, x2b[n], (14, 22))
                y0ps.append(y0_ps)
            x2fin = []
            for n in range(BPC):
                y0p = y0pads[(1, n)]
                nc.vector.tensor_scalar_mul(out=y0p[:, 2:16, 2:24], in0=y0ps[n][:],
                                            scalar1=s1_sb[:])
                z_ps = pp.tile([D, NT, 24], F32, tag="ps", bufs=6)
                conv9(z_ps, "w2T1", 0, y0p, (16, 24))
                x2h = ap.tile([D, NT, 24], BF16, tag="x2h", bufs=2)
                with nc.allow_low_precision(reason="head input, tol 2e-2"):
                    nc.vector.tensor_add(out=x2h[:], in0=z_ps[:], in1=x2[n][:])
                x2fin.append(x2h)

            # ---------------- heads ----------------
            o_all = ap.tile([PRED, BPC, COUT], F32, tag="oall")
            for n in range(BPC):
                x2f = x2fin[n][:].rearrange("p a b -> p (a b)")
                y1_ps = pp.tile([D, PRED], F32, tag="ps", bufs=6)
                for e in range(3):
                    nc.tensor.matmul(y1_ps[:], lhsT=x2f[:, 128 * e:128 * (e + 1)],
                                     rhs=pv("fc1e", PRED * e, PRED * (e + 1)),
                                     start=(e == 0), stop=False)
                nc.tensor.matmul(y1_ps[:], lhsT=ones_sb[:], rhs=pv("fc1b", 0, PRED),
                                 start=False, stop=True)
                y1_sb = ap.tile([D, PRED], BF16, tag="y1sb")
                nc.scalar.copy(out=y1_sb[:], in_=y1_ps[:])
                o_ps = pp.tile([PRED, COUT], F32, tag="ps", bufs=6)
                nc.tensor.matmul(o_ps[:], lhsT=y1_sb[:], rhs=pv("fc2T", 0, COUT),
                                 start=True, stop=False)
                nc.tensor.matmul(o_ps[:], lhsT=ones_sb[:, 0:PRED], rhs=pv("fc2b", 0, COUT),
                                 start=False, stop=True)
                (nc.vector.tensor_copy if n == 0 else
                 (lambda out, in_: nc.scalar.copy(out=out, in_=in_)))(
                    out=o_all[:, n, :], in_=o_ps[:])
            nc.sync.dma_start(out=out[:, :, :].rearrange("n p c -> p n c"),
                              in_=o_all[:])

    nc.finalize()
    return nc


def kernel(**inputs):
    global LAST_RESULT
    import ml_dtypes
    inputs = {k: np.ascontiguousarray(np.asarray(v, np.float32)) for k, v in inputs.items()}
    if "nc" not in _CACHE:
        _CACHE["nc"] = _build()
    nc = _CACHE["nc"]
    c = _prep_consts(
        inputs["token_w"], inputs["patch_w"], inputs["patch_b"], inputs["Wi"],
        inputs["pconv_w"], inputs["pconv_b"], inputs["bn_g"], inputs["bn_b"],
        inputs["aconv_w"], inputs["fc1_w"], inputs["fc1_b"], inputs["fc2_w"],
        inputs["fc2_b"])
    xtp_full = np.pad(inputs["x"].transpose(0, 2, 1), ((0, 0), (0, 0), (1, 1)),
                      mode="wrap").astype(np.float32)
    # im2col gathers of x for the stats path (identical on all cores)
    xG = np.empty((CIN, 14, B, NT), np.float32)
    for jp in range(14):
        xG[:, jp] = xtp_full[:, :, jp::12][:, :, :NT].transpose(1, 0, 2)
    xG = xG.reshape(CIN * 14, B * NT)
    c["xg"] = np.stack([xG[98 * i:98 * (i + 1)] for i in range(3)], 1).reshape(98, 3 * 256)
    xG2 = np.empty((CIN, 38, B, 14), np.float32)
    for jp in range(38):
        xG2[:, jp] = xtp_full[:, :, jp::12][:, :, :14].transpose(1, 0, 2)
    xG2 = xG2.reshape(CIN * 38, B * 14)
    c["xg2"] = np.stack([xG2[114 * i:114 * (i + 1)] for i in range(7)], 1).reshape(114, 7 * 224)

    base = {}
    for pname, (parts, dt, plist) in PACKS.items():
        cols = []
        for nm, pparts, w_ in plist:
            a = np.zeros((parts, w_), np.float32)
            a[:pparts] = np.asarray(c[nm], np.float32).reshape(pparts, w_)
            cols.append(a)
        arr = np.concatenate(cols, axis=1)
        base[pname] = np.ascontiguousarray(
            arr.astype(ml_dtypes.bfloat16 if dt == "bf16" else np.float32))
    in_maps = []
    for core in range(N_CORES):
        m = dict(base)
        xt = np.concatenate([xtp_full[BPC * core + n] for n in range(BPC)], axis=1)
        m["xtp"] = np.ascontiguousarray(xt.astype(ml_dtypes.bfloat16))
        in_maps.append(m)
    import os
    res = run_bass_kernel_spmd(nc, in_maps, core_ids=list(range(N_CORES)),
                               trace=bool(os.environ.get("BASS_TRACE")))
    LAST_RESULT = res
    return np.concatenate([res.results[cid]["out"] for cid in range(N_CORES)], axis=0)


# revision 21
# speedup vs baseline: 1.0707x; 1.0128x over previous
"""Self-contained Trainium2 Bass kernel for nn_Model_16801912062040 (dense_cnn).

Sharding: batch-parallel, 2 samples per core across 8 cores, ZERO collectives.
The dynamic conv collapses algebraically: y[n,(m,o)] = alpha[m,o]*(x2[n] (x) Wi[o])
and the conv_transpose contraction over (m,o) reduces to
    z[n,i] = sum_o s_o * (Y0[n,o] (x)_full flip(Wi[o,i])),  s_o = sum_m alpha[m,o]^2
so only the per-channel scalar s (128 floats per layer) couples samples.

Every core computes s for BOTH layers locally from the full x: all pooled
statistics are linear in x up to the alpha nonlinearity, and the raster reshape
(B,L,D)->(B*D,1,L) has an exact shift structure in nt (1536 flat elements = 12
l-steps), so pp0 and the Y0 row-sums (ry0) for all 16 samples are stride-12
convs of x with host-composed kernels (G0: 14 taps; G2: 38 taps folding Wi and
the windowed-column-sum edge corrections); row-sums of z then come from 3
matmuls against row-flipped row-summed Wi (wrf), giving pp1 and s1. Each core
is fully independent: no barriers, no skew amplification.

Perf notes:
- fp32 matmuls run ~4x slower per column than bf16 -> every matmul input is
  bf16 (PSUM accumulation stays f32).
- Each dma_start costs ~700ns serialized on its engine's sequencer, so params
  are packed into 6 loads spread across different engine queues.
- The stats pipeline is interleaved between conv blocks so its cross-engine
  latency hides behind matmul streams instead of stalling the PE queue.
- The psum->x2 gather is 6 strided AP copies instead of 24 per sample.
"""
import math
import numpy as np

import concourse.bass as bass
import concourse.tile as tile
from concourse import bacc, mybir
from concourse.bass_utils import run_bass_kernel_spmd

N_CORES = 8
BPC = 2            # samples per core
B, L, CIN, D = 16, 192, 21, 128
P, S, NT, PRED, COUT = 24, 12, 16, 96, 21
LAYERS = 2
BN_EPS = 1e-5
F32 = mybir.dt.float32
BF16 = mybir.dt.bfloat16

# pack -> (partitions, dtype, [(piece, piece_partitions, cols), ...])
PACKS = {
    "pkm": (128, "bf16", [("posx2", 128, 384),
                          ("w2pa", 128, 384), ("w2pb", 65, 384)]),
    "pkw1": (128, "bf16", [("wiT0", 128, 1152), ("w2T0", 128, 1152)]),
    "pkw2": (128, "bf16", [("wiT1", 128, 1152), ("w2T1", 128, 1152)]),
    "pkt": (21, "bf16", [("tokA", 21, 384), ("tokR", 21, 384)]),
    "pkc": (128, "bf16", [("bns", 128, 2), ("bnb", 128, 2),
                          ("cpp0r", 128, 256), ("c2r", 128, 224),
                          ("pcvT", 128, 768), ("awT", 128, 256),
                          ("wrf", 128, 384), ("fc1e", 128, 288),
                          ("fc2T", 128, 21), ("g0T", 98, 384), ("xg", 98, 768),
                          ("g2T", 114, 896), ("xg2", 114, 1568),
                          ("fc1b", 1, 96), ("fc2b", 1, 21)]),
}

_CACHE = {}
LAST_RESULT = None


def _pos_embed():
    pos = np.arange(L, dtype=np.float32)[:, None]
    div = np.exp(np.arange(0, D, 2, dtype=np.float32) * (-math.log(10000.0) / D))
    pe = np.zeros((L, D), np.float32)
    pe[:, 0::2] = np.sin(pos * div)
    pe[:, 1::2] = np.cos(pos * div)
    return pe


def _slice_map(p, patch_w2, patch_b):
    """Phi/weight/bias map for xe4[:, nt, p, d_t] in E-flat space (nt-shift-free)."""
    d_t = np.arange(D)
    g = p // 3
    inner = (p % 3) * 128 + d_t
    nts = inner // 24
    ps = inner % 24
    k = np.arange(P)
    j = np.minimum(12 * nts[:, None] + k[None, :], 191)   # edge-pad fold
    Phi = 192 * g + j
    return Phi, patch_w2[ps], patch_b[ps]


def _density(ps_list, patch_w2, patch_b):
    d_t = np.arange(D)
    dens = np.zeros((D, 12, 128), np.float32)
    bias = np.zeros(D, np.float32)
    for p in ps_list:
        Phi, w, b = _slice_map(p, patch_w2, patch_b)
        for k in range(P):
            np.add.at(dens, (d_t, Phi[:, k] // 128, Phi[:, k] % 128), w[:, k])
        bias += b
    return dens, bias


def _compose_G(dens, bias, token_w, pe_t):
    G = np.zeros((D, CIN, 14), np.float32)
    for k2 in range(3):
        G[:, :, k2:k2 + 12] += np.einsum("dlm,mc->dcl", dens, token_w[:, :, k2])
    C = np.einsum("dlm,tlm->dt", dens, pe_t) + bias[:, None]
    return G, C


def _prep_consts(token_w, patch_w, patch_b, Wi, pconv_w, pconv_b, bn_g, bn_b,
                 aconv_w, fc1_w, fc1_b, fc2_w, fc2_b):
    c = {}
    c["tokA"] = token_w.transpose(1, 2, 0).reshape(CIN, -1)
    c["tokR"] = np.roll(token_w, -64, 0).transpose(1, 2, 0).reshape(CIN, -1)
    pe = _pos_embed()
    pos_flat = pe.reshape(L * D)
    xe4_pos = np.zeros((NT, P, D), np.float32)
    for p_ in range(P):
        Phi, w, _ = _slice_map(p_, patch_w[:, 0, :], patch_b)
        for nt in range(NT):
            xe4_pos[nt, p_] = (pos_flat[1536 * nt + Phi] * w).sum(-1)
    wmap = 2 * (np.arange(24) % 12) + np.arange(24) // 12     # p(w)
    posx2 = xe4_pos[:, wmap, :].transpose(2, 0, 1)            # [d, t, w]
    c["posx2"] = posx2.reshape(D, NT * 24)
    W2p = np.zeros((205, 384), np.float32)
    for nt in range(NT):
        for p_ in range(P):
            j = nt * 24 + p_
            for k in range(P):
                m = 12 * nt + k
                W2p[min(m, 191), j] += patch_w[p_, 0, k]   # replicate-pad fold
            W2p[204, j] = patch_b[p_]
    c["w2pa"] = W2p[0:128]
    c["w2pb"] = np.concatenate([W2p[128:192], W2p[204:205]], 0)
    A = Wi.transpose(0, 3, 4, 2, 1).reshape(LAYERS, 9, D, D)                  # [l,tap,i,o]
    wiT = A.transpose(2, 0, 1, 3).reshape(D, LAYERS * 9 * D)
    c["wiT0"], c["wiT1"] = wiT[:, :9 * D], wiT[:, 9 * D:]
    Wf = Wi[:, :, :, ::-1, ::-1]
    Bt = Wf.transpose(0, 3, 4, 1, 2).reshape(LAYERS, 9, D, D)                 # [l,tap,o,i]
    w2T = Bt.transpose(2, 0, 1, 3).reshape(D, LAYERS * 9 * D)
    c["w2T0"], c["w2T1"] = w2T[:, :9 * D], w2T[:, 9 * D:]
    Ct = pconv_w.transpose(0, 3, 2, 1) / 24.0
    c["pcvT"] = Ct.transpose(2, 0, 1, 3).reshape(D, LAYERS * 3 * D)
    c["awT"] = (aconv_w.transpose(2, 0, 1) / 16.0).reshape(D, LAYERS * D)
    c["bns"] = (bn_g / np.sqrt(1.0 + BN_EPS)).T
    c["fc1e"] = fc1_w.T.reshape(D, 3 * PRED)
    c["fc1b"] = fc1_b[None, :]
    c["fc2T"] = fc2_w.T
    c["fc2b"] = fc2_b[None, :]
    c["bnb"] = (pconv_b * (bn_g / np.sqrt(1.0 + BN_EPS)) + bn_b).T

    # ---- composed all-sample stats kernels (no collectives) ----
    patch_w2 = patch_w[:, 0, :]
    pe_t = pe[(12 * np.arange(NT)[:, None] + np.arange(12)[None, :]).reshape(-1)].reshape(NT, 12, D)
    dens0, bias0 = _density(range(P), patch_w2, patch_b)
    G0, C0 = _compose_G(dens0, bias0, token_w, pe_t)
    dxe = {p: _density([p], patch_w2, patch_b) for p in (0, 2, 21, 23)}
    cxd = [
        (dens0 - dxe[21][0] - dxe[23][0], bias0 - dxe[21][1] - dxe[23][1]),
        (dens0 - dxe[0][0] - dxe[23][0], bias0 - dxe[0][1] - dxe[23][1]),
        (dens0 - dxe[0][0] - dxe[2][0], bias0 - dxe[0][1] - dxe[2][1]),
    ]
    G2 = np.zeros((D, CIN, 38), np.float32)
    C2 = np.zeros((D, 14), np.float32)
    for dq in range(3):
        Gcx, Ccx = _compose_G(cxd[dq][0], cxd[dq][1], token_w, pe_t)
        for dp in range(3):
            Wt = Wi[0][:, :, dp, dq]
            G2[:, :, 12 * dp:12 * dp + 14] += np.einsum("oi,icj->ocj", Wt, Gcx)
            C2 += np.einsum("oi,it->ot", Wt, Ccx[:, dp:dp + 14])
    G0T = G0.reshape(D, CIN * 14).T
    c["g0T"] = np.stack([G0T[98 * i:98 * (i + 1)] for i in range(3)], 1).reshape(98, 3 * 128)
    G2T = G2.reshape(D, CIN * 38).T
    c["g2T"] = np.stack([G2T[114 * i:114 * (i + 1)] for i in range(7)], 1).reshape(114, 7 * 128)
    c["cpp0r"] = np.broadcast_to(C0[:, None, :], (D, B, NT)).reshape(D, -1)
    c["c2r"] = np.broadcast_to(C2[:, None, :], (D, B, 14)).reshape(D, -1)
    wrfn = Wi[0][:, :, ::-1, :].sum(-1)                                       # [o, i, dp]
    c["wrf"] = wrfn.transpose(0, 2, 1).reshape(D, 3 * D)
    return c


def _build():
    nc = bacc.Bacc("TRN2", target_bir_lowering=False, debug=False, num_devices=N_CORES)

    xtp = nc.declare_dram_parameter("xtp", [CIN, BPC * 194], BF16, isOutput=False)
    pk_params = {}
    for pname, (parts, dt, pieces) in PACKS.items():
        total = sum(w for _, _, w in pieces)
        pk_params[pname] = nc.declare_dram_parameter(
            pname, [parts, total], F32 if dt == "f32" else BF16, isOutput=False)
    out = nc.declare_dram_parameter("out", [BPC, PRED, COUT], F32, isOutput=True)

    RELU = mybir.ActivationFunctionType.Relu
    SQUARE = mybir.ActivationFunctionType.Square
    ADD = mybir.AluOpType.add
    AX = mybir.AxisListType.X

    with tile.TileContext(nc) as tc:
        with tc.tile_pool(name="w", bufs=1) as wp, \
             tc.tile_pool(name="act", bufs=2) as ap, \
             tc.tile_pool(name="x2p", bufs=6) as xp, \
             tc.tile_pool(name="ps", bufs=1, space="PSUM") as pp:

            # --- packed param loads, triggers spread across engine queues ---
            pieces = {}
            dma_eng = {"pkt": nc.sync, "pkm": nc.scalar, "pkw1": nc.scalar}
            xt_sb = wp.tile([CIN, BPC * 194], BF16, tag="xt")
            nc.sync.dma_start(out=xt_sb[:], in_=xtp[:, :])
            for pname in ("pkt", "pkm", "pkw1", "pkc", "pkw2"):
                parts, dt, plist = PACKS[pname]
                total = sum(w for _, _, w in plist)
                t = wp.tile([parts, total], F32 if dt == "f32" else BF16, tag=pname)
                if pname not in ("pkw2", "pkc"):
                    dma_eng[pname].dma_start(out=t[:], in_=pk_params[pname][:, :])
                o = 0
                for nm, pparts, w_ in plist:
                    pieces[nm] = (t, pparts, o)
                    o += w_
            pkw2_tile = pieces["wiT1"][0]
            pkc_tile = pieces["g0T"][0]
            pkm_tile = pieces["w2pa"][0]
            # Sequential DMA waves: corner-copies create WAW deps that keep
            # each wave off the byte-bound DMA rings until the prior wave
            # lands (rings round-robin at line granularity, so concurrent big
            # packs starve urgent small ones).
            nc.scalar.copy(out=pkc_tile[0:1, 0:1], in_=pkm_tile[0:1, 0:1])
            nc.scalar.dma_start(out=pkc_tile[:], in_=pk_params["pkc"][:, :])
            nc.vector.tensor_copy(out=pkw2_tile[0:1, 0:1], in_=pkc_tile[0:1, 0:1])
            nc.sync.dma_start(out=pkw2_tile[:], in_=pk_params["pkw2"][:, :])

            def pv(nm, a, b_):
                t, pparts, o = pieces[nm]
                return t[0:pparts, o + a:o + b_]

            bnsb = wp.tile([D, 4], F32, tag="bnsb")
            ones_sb = wp.tile([1, D], BF16, tag="ones")
            nc.vector.memset(ones_sb[:], 1.0)
            ryspad = wp.tile([D, B, 18], BF16, tag="ryspad")
            nc.vector.memset(ryspad[:], 0.0)
            y0pads = {}
            for l in range(LAYERS):
                for n in range(BPC):
                    y0p = ap.tile([D, 18, 26], BF16, tag="y0p", bufs=4)
                    nc.vector.memset(y0p[:], 0.0)   # borders only matter
                    y0pads[(l, n)] = y0p

            # ---------------- embedding (own 2 samples) ----------------
            x2 = []
            for n in range(BPC):
                embs = []
                for tok in ("tokA", "tokR"):
                    e_ps = pp.tile([D, L], F32, tag="ps", bufs=6)
                    for k in range(3):
                        nc.tensor.matmul(e_ps[:], lhsT=pv(tok, k * D, (k + 1) * D),
                                         rhs=xt_sb[:, 194 * n + k:194 * n + k + L],
                                         start=(k == 0), stop=(k == 2))
                    embs.append(e_ps)
                eT = embs[0][:].rearrange("p (s c) -> p s c", c=3)
                eR = embs[1][:].rearrange("p (s c) -> p s c", c=3)
                xeA = ap.tile([D, 64, 2], BF16, tag="xeA")
                xeB = ap.tile([65, 64, 2], BF16, tag="xeB")
                nc.vector.tensor_copy(out=xeA[:, :, 0], in_=eT[:, :, 0])
                nc.vector.tensor_copy(out=xeA[0:64, :, 1], in_=eR[0:64, :, 1])
                nc.vector.tensor_copy(out=xeA[64:128, :, 1], in_=eR[64:128, :, 2])
                nc.scalar.copy(out=xeB[0:64, :, 0], in_=eT[0:64, :, 1])
                nc.scalar.copy(out=xeB[0:64, :, 1], in_=eR[0:64, :, 2])
                nc.vector.memset(xeB[64:65, :, :], 1.0)
                pcs = []
                for e in range(3):
                    pc_ps = pp.tile([D, NT, 8], F32, tag="ps", bufs=6)
                    nc.tensor.matmul(pc_ps[:], lhsT=pv("w2pa", 128 * e, 128 * (e + 1)),
                                     rhs=xeA[:], start=True, stop=False)
                    nc.tensor.matmul(pc_ps[:], lhsT=pv("w2pb", 128 * e, 128 * (e + 1)),
                                     rhs=xeB[:], start=False, stop=True)
                    pcs.append(pc_ps)
                # psum -> x2 permutation as 6 strided AP copies:
                # x2 w = 3k+r <-> pcs[e] q = 2k2+r2 with p(w) = 2*(w%12) + w//12
                x2n = xp.tile([D, NT, 24], F32, tag="x2")
                x2v = x2n[:].rearrange("p t (k r) -> p t k r", r=3)
                px2 = pv("posx2", 0, 384).rearrange("p (t k r) -> p t k r", k=8, r=3)
                plan = [(0, 0, 0, 0), (0, 1, 1, 1),
                        (1, 2, 0, 1), (1, 0, 1, 0),
                        (2, 1, 0, 0), (2, 2, 1, 1)]
                for i, (e, r, half, r2) in enumerate(plan):
                    pcv = pcs[e][:].rearrange("p t (k q) -> p t k q", q=2)
                    eng = nc.vector.tensor_add
                    eng(out=x2v[:, :, 4 * half:4 * (half + 1), r],
                        in0=pcv[:, :, :, r2],
                        in1=px2[:, :, 4 * half:4 * (half + 1), r])
                x2.append(x2n)

            nc.vector.tensor_copy(out=bnsb[:], in_=pv("bns", 0, 4))

            # ------- stats pipeline pieces (interleaved with conv blocks) -------
            def ppc_pool(l, ppsb):
                ppc_ps = pp.tile([D, B, NT], F32, tag="st", bufs=2)
                for k in range(3):
                    o = (l * 3 + k) * D
                    if k == 0:
                        nc.tensor.matmul(ppc_ps[:, :, 1:NT], lhsT=pv("pcvT", o, o + D),
                                         rhs=ppsb[:, :, 0:NT - 1], start=True, stop=False)
                    elif k == 1:
                        nc.tensor.matmul(ppc_ps[:], lhsT=pv("pcvT", o, o + D),
                                         rhs=ppsb[:], start=False, stop=False)
                    else:
                        nc.tensor.matmul(ppc_ps[:, :, 0:NT - 1], lhsT=pv("pcvT", o, o + D),
                                         rhs=ppsb[:, :, 1:NT], start=False, stop=True)
                ppc_sb = ap.tile([D, B, NT], F32, tag="ppc_sb", bufs=2)
                nc.scalar.activation(out=ppc_sb[:], in_=ppc_ps[:], func=RELU,
                                     bias=bnsb[:, 2 + l:3 + l], scale=bnsb[:, l:l + 1])
                pooled = ap.tile([D, B], BF16, tag="pooled", bufs=2)
                with nc.allow_low_precision(reason="16-term pooled mean, tol 2e-2"):
                    nc.vector.tensor_reduce(out=pooled[:], in_=ppc_sb[:], axis=AX, op=ADD)
                return pooled

            def alpha_sq(l, pooled):
                al_ps = pp.tile([D, B], F32, tag="st", bufs=2)
                nc.tensor.matmul(al_ps[:], lhsT=pv("awT", l * D, (l + 1) * D),
                                 rhs=pooled[:], start=True, stop=True)
                asq = ap.tile([D, B], F32, tag="asq", bufs=2)
                s_sb = ap.tile([D, 1], F32, tag="s", bufs=2)
                nc.scalar.activation(out=asq[:], in_=al_ps[:], func=SQUARE,
                                     bias=1.0, scale=1.0, accum_out=s_sb[:])
                return s_sb

            def conv9(out_ps, wname, base, rhs_tile, windows):
                for dp in range(3):
                    for dq in range(3):
                        tap = 3 * dp + dq
                        o = tap * D
                        nc.tensor.matmul(out_ps[:], lhsT=pv(wname, o, o + D),
                                         rhs=rhs_tile[:, dp:dp + windows[0],
                                                      dq:dq + windows[1]],
                                         start=(tap == 0), stop=(tap == 8))

            # stats phase 1: pp0, ry0, ppc0 -> pooled0
            pp0_ps = pp.tile([D, B, NT], F32, tag="st", bufs=2)
            for cc in range(3):
                nc.tensor.matmul(pp0_ps[:], lhsT=pv("g0T", 128 * cc, 128 * (cc + 1)),
                                 rhs=pv("xg", 256 * cc, 256 * (cc + 1)).rearrange(
                                     "p (b t) -> p b t", t=NT),
                                 start=(cc == 0), stop=(cc == 2))
            pp0_sb = wp.tile([D, B, NT], BF16, tag="pp0")
            nc.vector.tensor_add(out=pp0_sb[:], in0=pp0_ps[:],
                                 in1=pv("cpp0r", 0, 256).rearrange("p (b t) -> p b t", t=NT))
            ry0_ps = pp.tile([D, B, 14], F32, tag="st", bufs=2)
            for cc in range(7):
                nc.tensor.matmul(ry0_ps[:], lhsT=pv("g2T", 128 * cc, 128 * (cc + 1)),
                                 rhs=pv("xg2", 224 * cc, 224 * (cc + 1)).rearrange(
                                     "p (b r) -> p b r", r=14),
                                 start=(cc == 0), stop=(cc == 6))
            ry0_sb = ap.tile([D, B, 14], F32, tag="ry0")
            nc.vector.tensor_add(out=ry0_sb[:], in0=ry0_ps[:],
                                 in1=pv("c2r", 0, 224).rearrange("p (b r) -> p b r", r=14))
            pooled0 = ppc_pool(0, pp0_sb)

            # conv l=0 part A: casts + Y0
            x2b = []
            for n in range(BPC):
                x2bn = ap.tile([D, NT, 24], BF16, tag="x2b", bufs=2)
                (nc.vector.tensor_copy if n == 0 else
                 (lambda out, in_: nc.scalar.copy(out=out, in_=in_)))(
                    out=x2bn[:], in_=x2[n][:])
                x2b.append(x2bn)
            y0ps = []
            for n in range(BPC):
                y0_ps = pp.tile([D, 14, 22], F32, tag="ps", bufs=6)
                conv9(y0_ps, "wiT0", 0, x2b[n], (14, 22))
                y0ps.append(y0_ps)

            # stats phase 2: s0, rsz -> pp1 -> pooled1
            s0_sb = alpha_sq(0, pooled0)
            nc.vector.tensor_scalar_mul(out=ryspad[:, :, 2:16], in0=ry0_sb[:],
                                        scalar1=s0_sb[:])
            rsz_ps = pp.tile([D, B, NT], F32, tag="st", bufs=2)
            for dp in range(3):
                nc.tensor.matmul(rsz_ps[:], lhsT=pv("wrf", dp * D, (dp + 1) * D),
                                 rhs=ryspad[:, :, dp:dp + 16],
                                 start=(dp == 0), stop=(dp == 2))
            pp1_sb = wp.tile([D, B, NT], BF16, tag="pp1")
            nc.vector.tensor_add(out=pp1_sb[:], in0=rsz_ps[:], in1=pp0_sb[:])
            pooled1 = ppc_pool(1, pp1_sb)

            # conv l=0 part B: scale + z + residual
            for n in range(BPC):
                y0p = y0pads[(0, n)]
                nc.vector.tensor_scalar_mul(out=y0p[:, 2:16, 2:24], in0=y0ps[n][:],
                                            scalar1=s0_sb[:])
                z_ps = pp.tile([D, NT, 24], F32, tag="ps", bufs=6)
                conv9(z_ps, "w2T0", 0, y0p, (16, 24))
                x2n = xp.tile([D, NT, 24], F32, tag="x2")
                nc.vector.tensor_add(out=x2n[:], in0=z_ps[:], in1=x2[n][:])
                x2[n] = x2n

            s1_sb = alpha_sq(1, pooled1)

            # conv l=1
            x2b = []
            for n in range(BPC):
                x2bn = ap.tile([D, NT, 24], BF16, tag="x2b", bufs=2)
                (nc.vector.tensor_copy if n == 0 else
                 (lambda out, in_: nc.scalar.copy(out=out, in_=in_)))(
                    out=x2bn[:], in_=x2[n][:])
                x2b.append(x2bn)
            y0ps = []
            for n in range(BPC):
                y0_ps = pp.tile([D, 14, 22], F32, tag="ps", bufs=6)
                conv9(y0_ps, "wiT1", 0, x2b[n], (14, 22))
                y0ps.append(y0_ps)
            x2fin = []
            for n in range(BPC):
                y0p = y0pads[(1, n)]
                nc.vector.tensor_scalar_mul(out=y0p[:, 2:16, 2:24], in0=y0ps[n][:],
                                            scalar1=s1_sb[:])
                z_ps = pp.tile([D, NT, 24], F32, tag="ps", bufs=6)
                conv9(z_ps, "w2T1", 0, y0p, (16, 24))
                x2h = ap.tile([D, NT, 24], BF16, tag="x2h", bufs=2)
                with nc.allow_low_precision(reason="head input, tol 2e-2"):
                    nc.vector.tensor_add(out=x2h[:], in0=z_ps[:], in1=x2[n][:])
                x2fin.append(x2h)

            # ---------------- heads ----------------
            o_all = ap.tile([PRED, BPC, COUT], F32, tag="oall")
            for n in range(BPC):
                x2f = x2fin[n][:].rearrange("p a b -> p (a b)")
                y1_ps = pp.tile([D, PRED], F32, tag="ps", bufs=6)
                for e in range(3):
                    nc.tensor.matmul(y1_ps[:], lhsT=x2f[:, 128 * e:128 * (e + 1)],
                                     rhs=pv("fc1e", PRED * e, PRED * (e + 1)),
                                     start=(e == 0), stop=False)
                nc.tensor.matmul(y1_ps[:], lhsT=ones_sb[:], rhs=pv("fc1b", 0, PRED),
                                 start=False, stop=True)
                y1_sb = ap.tile([D, PRED], BF16, tag="y1sb")
                nc.scalar.copy(out=y1_sb[:], in_=y1_ps[:])
                o_ps = pp.tile([PRED, COUT], F32, tag="ps", bufs=6)
                nc.tensor.matmul(o_ps[:], lhsT=y1_sb[:], rhs=pv("fc2T", 0, COUT),
                                 start=True, stop=False)
                nc.tensor.matmul(o_ps[:], lhsT=ones_sb[:, 0:PRED], rhs=pv("fc2b", 0, COUT),
                                 start=False, stop=True)
                (nc.vector.tensor_copy if n == 0 else
                 (lambda out, in_: nc.scalar.copy(out=out, in_=in_)))(
                    out=o_all[:, n, :], in_=o_ps[:])
            nc.sync.dma_start(out=out[:, :, :].rearrange("n p c -> p n c"),
                              in_=o_all[:])

    nc.finalize()
    return nc


def kernel(**inputs):
    global LAST_RESULT
    import ml_dtypes
    inputs = {k: np.ascontiguousarray(np.asarray(v, np.float32)) for k, v in inputs.items()}
    if "nc" not in _CACHE:
        _CACHE["nc"] = _build()
    nc = _CACHE["nc"]
    c = _prep_consts(
        inputs["token_w"], inputs["patch_w"], inputs["patch_b"], inputs["Wi"],
        inputs["pconv_w"], inputs["pconv_b"], inputs["bn_g"], inputs["bn_b"],
        inputs["aconv_w"], inputs["fc1_w"], inputs["fc1_b"], inputs["fc2_w"],
        inputs["fc2_b"])
    xtp_full = np.pad(inputs["x"].transpose(0, 2, 1), ((0, 0), (0, 0), (1, 1)),
                      mode="wrap").astype(np.float32)
    # im2col gathers of x for the stats path (identical on all cores)
    xG = np.empty((CIN, 14, B, NT), np.float32)
    for jp in range(14):
        xG[:, jp] = xtp_full[:, :, jp::12][:, :, :NT].transpose(1, 0, 2)
    xG = xG.reshape(CIN * 14, B * NT)
    c["xg"] = np.stack([xG[98 * i:98 * (i + 1)] for i in range(3)], 1).reshape(98, 3 * 256)
    xG2 = np.empty((CIN, 38, B, 14), np.float32)
    for jp in range(38):
        xG2[:, jp] = xtp_full[:, :, jp::12][:, :, :14].transpose(1, 0, 2)
    xG2 = xG2.reshape(CIN * 38, B * 14)
    c["xg2"] = np.stack([xG2[114 * i:114 * (i + 1)] for i in range(7)], 1).reshape(114, 7 * 224)

    base = {}
    for pname, (parts, dt, plist) in PACKS.items():
        cols = []
        for nm, pparts, w_ in plist:
            a = np.zeros((parts, w_), np.float32)
            a[:pparts] = np.asarray(c[nm], np.float32).reshape(pparts, w_)
            cols.append(a)
        arr = np.concatenate(cols, axis=1)
        base[pname] = np.ascontiguousarray(
            arr.astype(ml_dtypes.bfloat16 if dt == "bf16" else np.float32))
    in_maps = []
    for core in range(N_CORES):
        m = dict(base)
        xt = np.concatenate([xtp_full[BPC * core + n] for n in range(BPC)], axis=1)
        m["xtp"] = np.ascontiguousarray(xt.astype(ml_dtypes.bfloat16))
        in_maps.append(m)
    import os
    res = run_bass_kernel_spmd(nc, in_maps, core_ids=list(range(N_CORES)),
                               trace=bool(os.environ.get("BASS_TRACE")))
    LAST_RESULT = res
    return np.concatenate([res.results[cid]["out"] for cid in range(N_CORES)], axis=0)
